# revision 1
# baseline (speedup 1.0000x reference)
"""Trainium2 Bass kernel for nn_Attention_50216757625003.

GQA attention layer: B=2, S=1024, D=4096, H=32 q-heads, KV=8 kv-heads,
hd=128, A=10 gated adapter tokens, RoPE, split softmax (adapter block
softmaxed separately and scaled by tanh(gate)), causal mask.

Sharding (8 NeuronCores): outer data-parallel over batch (2) x
tensor-parallel over heads (4 groups of 8 q-heads / 2 kv-heads).
wq/wk/wv are sharded column-wise, wo row-wise; each core computes a
partial [S, D] output contribution and the host sums the 4 head-group
partials per batch element.

Device-side layout tricks:
  * x is fed transposed ([D, S]) so all projections run with D on the
    contraction (partition) axis.
  * q/k head dims are permuted even-first on the host (wq/wk column
    permutation); RoPE pairs then live on partitions p and p+64.  A
    cheap SBUF->SBUF DMA swaps the halves so the rotation becomes four
    partition-aligned DVE ops against duplicated cos/sin tables.
  * scores are built transposed ([keys, q]) so softmax denominators come
    from a ones-vector matmul and probs feed the PV matmul directly (no
    transposes anywhere).
  * softmax skips the max-subtraction (scores are O(1) here; exp is safe
    in fp32), which the per-block normalization keeps exact.
  * matmuls run in float32r (full PE rate; true fp32 is quarter rate).
    All matmul operands are materialized as f32r tiles so the walrus
    verifier sees rounded producers.  KMM env switches "f32r"/"f32"/
    "bf16".
"""

import os
import sys

import numpy as np

for _p in ("/opt/trn_rl_repo",):
    if _p not in sys.path and os.path.isdir(_p):
        sys.path.insert(0, _p)

import concourse.bass as bass
import concourse.mybir as mybir
from concourse import bacc
import concourse.tile as tile
from concourse.bass_utils import run_bass_kernel_spmd

HD = 128  # head dim (hardcoded: rope split + tile shapes assume 128)
A = 10    # adapter tokens
F32 = mybir.dt.float32

MM_MODE = os.environ.get("KMM", "f32r")

_PROG_CACHE = {}


def _md(mm):
    return {"f32r": mybir.dt.float32r, "f32": mybir.dt.float32,
            "bf16": mybir.dt.bfloat16}[mm]


# --------------------------------------------------------------------------
# device program
# --------------------------------------------------------------------------

def build_program(KO, S, HL, KVL, causal, mm):
    """One NeuronCore's program.

    KO: D // 128 contraction chunks.  S: sequence length.  HL: q heads on
    this core.  KVL: kv heads on this core.  causal: hardwire causal
    masking (tri mask on diagonal chunks + chunk skipping); otherwise an
    additive mask [S, S] is an input.  mm: matmul operand dtype mode.
    """
    nc = bacc.Bacc(None, target_bir_lowering=False,
                   dynamic_dma_scratch_size=2048)
    MD = _md(mm)
    D = KO * 128
    QB = min(512, S)       # q column block (psum bank + fp32 moving max)
    NQH = S // QB
    KC = S // 128          # token key chunks
    SA = S + A
    nrep = HL // KVL

    xp = nc.declare_dram_parameter("xp", [128, KO, S], MD, isOutput=False)
    wqp = nc.declare_dram_parameter("wqp", [HL, 128, KO, HD], MD, isOutput=False)
    wkp = nc.declare_dram_parameter("wkp", [KVL, 128, KO, HD], MD, isOutput=False)
    wvp = nc.declare_dram_parameter("wvp", [128, KO, KVL * HD], MD, isOutput=False)
    wop = nc.declare_dram_parameter("wop", [128, HL, D], MD, isOutput=False)
    adp = nc.declare_dram_parameter("adp", [128, KO, A], MD, isOutput=False)
    csp = nc.declare_dram_parameter("csp", [128, 2, S], F32, isOutput=False)
    trip = nc.declare_dram_parameter("trip", [128, 128], MD, isOutput=False)
    gcp = nc.declare_dram_parameter("gcp", [1, HL * 128], MD, isOutput=False)
    if not causal:
        mtp = nc.declare_dram_parameter("mtp", [128, KC, S], F32, isOutput=False)
    outp = nc.declare_dram_parameter("out", [S // 128, 128, D], F32, isOutput=True)
    HSPLIT = 1
    if HSPLIT == 2:
        outp2 = nc.declare_dram_parameter("out2", [S // 128, 128, D], F32,
                                          isOutput=True)

    Exp = mybir.ActivationFunctionType.Exp

    with tile.TileContext(nc) as tc:
        with tc.tile_pool(name="singles", bufs=1) as singles, \
             tc.tile_pool(name="persist", bufs=1) as persist:
            tri = singles.tile([128, 128], MD)
            nc.sync.dma_start(tri, trip[:])

            # all-ones vectors: row 0 / column 127 of the tri mask
            ones_row = tri[0:1, :]
            ones_col = tri[:, 127:128]
            zb = singles.tile([128, 1], F32)
            nc.vector.memset(zb, 0.0)

            # resident x^T in XG-chunk tiles, DMA'd just-in-time from the
            # V-projection loop so the first matmuls start early
            XG = min(4, KO)
            NX = KO // XG
            xt = [persist.tile([128, XG, S], MD, tag=f"x{i}", name=f"x{i}")
                  for i in range(NX)]
            xt_loaded = [False] * NX

            def xload(i):
                if not xt_loaded[i]:
                    h = XG // 2 or 1
                    nc.sync.dma_start(xt[i][:, 0:h, :],
                                      xp[:, i * XG:i * XG + h, :])
                    if h < XG:
                        nc.sync.dma_start(xt[i][:, h:XG, :],
                                          xp[:, i * XG + h:(i + 1) * XG, :])
                    xt_loaded[i] = True

            def xsl(c):
                return xt[c // XG][:, c % XG, :]

            kT = [persist.tile([128, SA], MD, tag=f"kT{j}", name=f"kT{j}")
                  for j in range(KVL)]
            vv = persist.tile([128, KC + 1, KVL * HD], MD, tag="vv")
            qT = [persist.tile([128, S], MD, tag=f"qT{h}", name=f"qT{h}")
                  for h in range(HL)]

            # ---------------- phase 1: projections -----------------------
            with tc.tile_pool(name="wpool", bufs=3) as wpool, \
                 tc.tile_pool(name="rpool", bufs=2) as rpool, \
                 tc.tile_pool(name="cpool", bufs=1) as cpool:
                # csA: cos^T duplicated on both partition halves; csB: sin^T
                csd = cpool.tile([128, 2, S], F32)
                nc.sync.dma_start(csd, csp[:])
                csA = csd[:, 0, :]
                csB = csd[:, 1, :]
                adT = cpool.tile([128, KO, A], MD)
                nc.sync.dma_start(adT, adp[:])

                def emit_rope(ps_h, dst, hh):
                    # psum rows 0:64 = x0 (even pair elems), 64:128 = x1.
                    # dst[0:64] = x0*cos - x1*sin ; dst[64:128] = x0*sin + x1*cos
                    sl = slice(hh * QB, (hh + 1) * QB)
                    rc = rpool.tile([128, QB], F32, tag="rc", bufs=2)
                    nc.scalar.copy(rc, ps_h)        # frees the psum slot fast
                    rs = rpool.tile([128, QB], F32, tag="rs", bufs=2)
                    nc.sync.dma_start(rs[0:64, :], rc[64:128, :])
                    nc.sync.dma_start(rs[64:128, :], rc[0:64, :])
                    # tm1 = [x0*cos ; x1*cos], tm2 = [x1*sin ; x0*sin]
                    tm1 = rpool.tile([128, QB], F32, tag="tm1", bufs=1)
                    tm2 = rpool.tile([128, QB], F32, tag="tm2", bufs=1)
                    nc.vector.tensor_mul(tm1, rc, csA[:, sl])
                    nc.vector.tensor_mul(tm2, rs, csB[:, sl])
                    nc.vector.tensor_sub(dst[0:64, sl], tm1[0:64, :], tm2[0:64, :])
                    nc.vector.tensor_add(dst[64:128, sl], tm2[64:128, :],
                                         tm1[64:128, :])

                ps1cm = tc.tile_pool(name="ps1", bufs=1, space="PSUM")
                ps1 = ps1cm.__enter__()

                # ---- V projection (token-major): stream wv once; accumulate
                # the cross-block partials in an f32 SBUF tile so psum needs
                # only 2 banks.  Adapter V accumulates in its own bank.
                WBV = min(4, KO)
                NVB = KO // WBV
                vacc = cpool.tile([128, KC, KVL * HD], F32)
                pav = ps1.tile([A, KVL * HD], F32, tag="av")

                def emit_vblock(b):
                    wt = wpool.tile([128, WBV, KVL * HD], MD, tag="w")
                    nc.sync.dma_start(wt, wvp[:, b * WBV:(b + 1) * WBV, :])
                    for i in range(b * WBV // XG,
                                   (b * WBV + WBV - 1) // XG + 1):
                        xload(i)
                    for t in range(KC):
                        psv = ps1.tile([128, KVL * HD], F32, tag="vproj",
                                       bufs=2)
                        for ci in range(WBV):
                            c = b * WBV + ci
                            nc.tensor.matmul(
                                psv[:, :], xsl(c)[:, t * 128:(t + 1) * 128],
                                wt[:, ci, :],
                                start=(ci == 0), stop=(ci == WBV - 1))
                        if b == 0 and NVB > 1:
                            nc.scalar.copy(vacc[:, t, :], psv[:, :])
                        elif b < NVB - 1:
                            nc.vector.tensor_add(vacc[:, t, :], vacc[:, t, :],
                                                 psv[:, :])
                        elif NVB > 1:
                            nc.vector.tensor_add(vv[:, t, :], vacc[:, t, :],
                                                 psv[:, :])
                        else:
                            nc.scalar.copy(vv[:, t, :], psv[:, :])
                    for ci in range(WBV):
                        c = b * WBV + ci
                        nc.tensor.matmul(pav[:, :], adT[:, c, :], wt[:, ci, :],
                                         start=(c == 0), stop=(c == KO - 1))
                    if b == NVB - 1:
                        nc.scalar.copy(vv[0:A, KC, :], pav[:, :])

                WBQ = min(8, KO)

                def emit_khead(j):
                    psk = [ps1.tile([128, QB], F32, tag="proj", bufs=4,
                                    name=f"psk{hh}") for hh in range(NQH)]
                    pak = ps1.tile([128, A], F32, tag="ak")
                    for b in range(KO // WBQ):
                        wt = wpool.tile([128, WBQ, HD], MD, tag="w")
                        nc.sync.dma_start(wt, wkp[j, :, b * WBQ:(b + 1) * WBQ, :])
                        for i in range(b * WBQ // XG,
                                       (b * WBQ + WBQ - 1) // XG + 1):
                            xload(i)
                        for ci in range(WBQ):
                            c = b * WBQ + ci
                            st, sp = (c == 0), (c == KO - 1)
                            for hh in range(NQH):
                                sl = slice(hh * QB, (hh + 1) * QB)
                                nc.tensor.matmul(
                                    psk[hh][:, :], wt[:, ci, :], xsl(c)[:, sl],
                                    start=st, stop=sp)
                            nc.tensor.matmul(
                                pak[:, :], wt[:, ci, :], adT[:, c, :],
                                start=st, stop=sp)
                    for hh in range(NQH):
                        emit_rope(psk[hh], kT[j], hh)
                    nc.scalar.copy(kT[j][:, S:SA], pak[:, 0:A])

                def emit_qhead(h):
                    psq = [ps1.tile([128, QB], F32, tag="proj", bufs=4,
                                    name=f"psq{hh}") for hh in range(NQH)]
                    for b in range(KO // WBQ):
                        wt = wpool.tile([128, WBQ, HD], MD, tag="w")
                        nc.sync.dma_start(wt, wqp[h, :, b * WBQ:(b + 1) * WBQ, :])
                        for i in range(b * WBQ // XG,
                                       (b * WBQ + WBQ - 1) // XG + 1):
                            xload(i)
                        for ci in range(WBQ):
                            c = b * WBQ + ci
                            st, sp = (c == 0), (c == KO - 1)
                            for hh in range(NQH):
                                sl = slice(hh * QB, (hh + 1) * QB)
                                nc.tensor.matmul(
                                    psq[hh][:, :], wt[:, ci, :], xsl(c)[:, sl],
                                    start=st, stop=sp)
                    for hh in range(NQH):
                        emit_rope(psq[hh], qT[h], hh)

                # Interleave V blocks between K/Q head projections so the
                # DMA-heavy V stream overlaps compute-heavy head projections.
                kq = [("k", j) for j in range(KVL)] + \
                     [("q", h) for h in range(HL)]
                vb = list(range(NVB))
                seq = []
                while vb or kq:
                    if vb:
                        seq.append(("v", vb.pop(0)))
                    if kq:
                        seq.append(kq.pop(0))
                for kind, idx in seq:
                    if kind == "v":
                        emit_vblock(idx)
                    elif kind == "k":
                        emit_khead(idx)
                    else:
                        emit_qhead(idx)
                ps1cm.__exit__(None, None, None)

            # ---------------- phase 2: attention --------------------------
            # oT / wo-weights / general-mask reuse the dead x-tile slots
            HG = min(4, HL)
            oTt = [persist.tile([128, HG, S], MD,
                                tag=(f"x{i}" if i < NX else f"oT{i}"),
                                name=f"oTall{i}")
                   for i in range((HL + HG - 1) // HG)]

            def oT(h):
                return oTt[h // HG][:, h % HG, :]

            mt = None
            if not causal:
                mtt = [persist.tile([128, KC // 2, S], F32,
                                    tag=(f"x{4 + i}" if NX > 5 else f"mt{i}"),
                                    name=f"mt{i}")
                       for i in range(2)]
                nc.sync.dma_start(mtt[0], mtp[:, 0:KC // 2, :])
                nc.sync.dma_start(mtt[1], mtp[:, KC // 2:KC, :])

                def mtsl(kc):
                    return mtt[kc // (KC // 2)][:, kc % (KC // 2), :]
            with tc.tile_pool(name="spool", bufs=3) as spool, \
                 tc.tile_pool(name="ps2", bufs=1, space="PSUM") as ps2:
                gc = spool.tile([1, HL * 128], MD, tag="gc", bufs=1)
                nc.sync.dma_start(gc, gcp[:])
                for h in range(HL):
                    j = h // nrep
                    for qh in range(NQH):
                        qs, qe = qh * QB, (qh + 1) * QB
                        if causal:
                            kcs = [kc for kc in range(KC) if kc * 128 < qe]
                        else:
                            kcs = list(range(KC))
                        ot_ps = ps2.tile([128, QB], F32, tag="ot", bufs=2)
                        oa_ps = ps2.tile([128, QB], F32, tag="oa", bufs=1)
                        dt_ps = ps2.tile([1, QB], F32, tag="dt", bufs=1)
                        da_ps = ps2.tile([1, QB], F32, tag="da", bufs=1)
                        for ki, kc in enumerate(kcs):
                            q0 = max(qs, kc * 128) if causal else qs
                            N = qe - q0
                            st, sp = (ki == 0), (ki == len(kcs) - 1)
                            scp = ps2.tile([128, QB], F32, tag="scp", bufs=2)
                            nc.tensor.matmul(
                                scp[:, 0:N],
                                kT[j][:, kc * 128:(kc + 1) * 128],
                                qT[h][:, q0:qe], start=True, stop=True)
                            pt = spool.tile([128, QB], MD, tag="pt", bufs=4)
                            if causal:
                                nc.scalar.activation(pt[:, 0:N], scp[:, 0:N],
                                                     Exp, bias=zb)
                                if kc * 128 >= qs:  # diagonal chunk
                                    nc.vector.tensor_mul(
                                        pt[:, 0:128], pt[:, 0:128], tri)
                            else:
                                sadd = spool.tile([128, QB], F32, tag="sadd",
                                                  bufs=2)
                                nc.vector.tensor_add(
                                    sadd[:, 0:N], scp[:, 0:N],
                                    mtsl(kc)[:, q0:qe])
                                nc.scalar.activation(pt[:, 0:N], sadd[:, 0:N],
                                                     Exp, bias=zb)
                            nc.tensor.matmul(
                                ot_ps[:, q0 - qs:QB],
                                vv[:, kc, j * HD:(j + 1) * HD],
                                pt[:, 0:N], start=st, stop=sp)
                            nc.tensor.matmul(
                                dt_ps[0:1, q0 - qs:QB], ones_col[:, 0:1],
                                pt[:, 0:N], start=st, stop=sp)
                        # adapter block
                        sca = ps2.tile([128, QB], F32, tag="scp", bufs=2)
                        nc.tensor.matmul(sca[0:A, :], kT[j][:, S:SA],
                                         qT[h][:, qs:qe], start=True, stop=True)
                        pa = spool.tile([128, QB], MD, tag="pt", bufs=4)
                        nc.scalar.activation(pa[0:A, :], sca[0:A, :], Exp,
                                             bias=zb[0:A, :])
                        nc.tensor.matmul(oa_ps[:, :],
                                         vv[0:A, KC, j * HD:(j + 1) * HD],
                                         pa[0:A, :], start=True, stop=True)
                        nc.tensor.matmul(da_ps[0:1, :], ones_col[0:A, 0:1],
                                         pa[0:A, :], start=True, stop=True)
                        # normalization factors (per-q scalars), f32r direct
                        rt = spool.tile([1, QB], MD, tag="rt", bufs=1)
                        ra = spool.tile([1, QB], MD, tag="ra", bufs=1)
                        with nc.allow_low_precision(
                                reason="f32r softmax scales, rounded like "
                                       "every other matmul operand"):
                            nc.vector.reciprocal(rt, dt_ps[0:1, :])
                            nc.vector.reciprocal(ra, da_ps[0:1, :])
                        # broadcast across partitions via rank-1 matmul;
                        # tanh(gate_h) is folded into the adapter lhsT (gc)
                        rp1 = ps2.tile([128, QB], F32, tag="rp", bufs=1)
                        nc.tensor.matmul(rp1, ones_row[0:1, :], rt[0:1, :],
                                         start=True, stop=True)
                        rtb = spool.tile([128, QB], F32, tag="rtb", bufs=1)
                        nc.scalar.copy(rtb, rp1)
                        rp2 = ps2.tile([128, QB], F32, tag="rp", bufs=1)
                        nc.tensor.matmul(rp2, gc[0:1, h * 128:(h + 1) * 128],
                                         ra[0:1, :], start=True, stop=True)
                        rab = spool.tile([128, QB], F32, tag="rab", bufs=1)
                        nc.scalar.copy(rab, rp2)
                        # oT = ot/denom_t + tanh(g)*oa/denom_a  (write-once)
                        tq1 = spool.tile([128, QB], F32, tag="tq1", bufs=1)
                        nc.vector.tensor_mul(tq1, ot_ps[:, :], rtb)
                        tq2 = spool.tile([128, QB], F32, tag="tq2", bufs=1)
                        nc.vector.tensor_mul(tq2, oa_ps[:, :], rab)
                        nc.vector.tensor_add(oT(h)[:, qs:qe], tq1, tq2)

            # ---------------- phase 3: output projection ------------------
            # Split into two half-head passes writing separate partial
            # outputs (host sums them): the first pass only needs heads
            # 0..HL/2-1 and overlaps the tail of the attention phase.
            with tc.tile_pool(name="wopool", bufs=2) as wopool, \
                 tc.tile_pool(name="obpool", bufs=3) as obpool, \
                 tc.tile_pool(name="ps3", bufs=4, space="PSUM") as ps3:
                NB = D // 512
                HH = HL // HSPLIT
                # deep prefetch of wo weight tiles into freed x slots
                slots = [i for i in range(2, NX)
                         if causal or i not in (4, 5)] or [None]
                for half in range(HSPLIT):
                    od = outp if half == 0 else outp2
                    h0 = half * HH
                    for n in range(NB):
                        wi = slots[(half * NB + n) % len(slots)]
                        if wi is not None and NX > wi:
                            won = persist.tile([128, HH, 512], MD,
                                               tag=f"x{wi}",
                                               name=f"won{half}_{n}")
                        else:
                            won = wopool.tile([128, HH, 512], MD, tag="won")
                        for wpi in range(0, HH, 2):
                            nc.sync.dma_start(
                                won[:, wpi:wpi + 2, :],
                                wop[:, h0 + wpi:h0 + wpi + 2,
                                    n * 512:(n + 1) * 512])
                        for m in range(S // 128):
                            pso = ps3.tile([128, 512], F32, tag="wo", bufs=6)
                            for hh2 in range(HH):
                                nc.tensor.matmul(
                                    pso,
                                    oT(h0 + hh2)[:, m * 128:(m + 1) * 128],
                                    won[:, hh2, :],
                                    start=(hh2 == 0), stop=(hh2 == HH - 1))
                            ob = obpool.tile([128, 512], F32, tag="ob")
                            nc.scalar.copy(ob, pso)
                            nc.sync.dma_start(
                                od[m, :, n * 512:(n + 1) * 512], ob)

    nc.compile()
    nc.finalize()
    return nc


def get_program(KO, S, HL, KVL, causal, mm):
    key = (KO, S, HL, KVL, causal, mm)
    if key not in _PROG_CACHE:
        _PROG_CACHE[key] = build_program(KO, S, HL, KVL, causal, mm)
    return _PROG_CACHE[key]


# --------------------------------------------------------------------------
# host-side sharding / layout prep
# --------------------------------------------------------------------------

_EVEN_FIRST = np.concatenate([np.arange(0, HD, 2), np.arange(1, HD, 2)])


def is_causal_mask(mask):
    S = mask.shape[-1]
    m = np.asarray(mask).reshape(S, S)
    iu = np.triu_indices(S, 1)
    il = np.tril_indices(S)
    return bool(np.all(m[il] == 0.0) and np.all(m[iu] <= -1e8))


def _np_md(mm):
    if mm == "bf16":
        import ml_dtypes
        return ml_dtypes.bfloat16
    return np.float32


def prep_core_inputs(core, G, x, wq, wk, wv, wo, adapter, gate,
                     freqs_cos, freqs_sin, mask, causal, mm=None):
    """Build the input dict for one core = (batch b, head-group g)."""
    mm = MM_MODE if mm is None else mm
    B, S, D = x.shape
    H = gate.shape[1]
    hd = wq.shape[1] // H
    KV = wk.shape[1] // hd
    KO = D // 128
    KC = S // 128
    HL, KVL = H // G, KV // G
    b, g = core // G, core % G
    hsl = slice(g * HL, (g + 1) * HL)
    ksl = slice(g * KVL, (g + 1) * KVL)
    idx = _EVEN_FIRST
    f32 = np.float32
    md = _np_md(mm)

    def c(a, dt=None):
        return np.ascontiguousarray(a, dtype=dt if dt is not None else md)

    xp = c(x[b].T.reshape(KO, 128, S).transpose(1, 0, 2))
    wq4 = wq.reshape(D, H, hd)[:, hsl][:, :, idx] * np.float32(1.0 / np.sqrt(hd))
    wqp = c(wq4.reshape(KO, 128, HL, hd).transpose(2, 1, 0, 3))
    wk4 = wk.reshape(D, KV, hd)[:, ksl][:, :, idx]
    wkp = c(wk4.reshape(KO, 128, KVL, hd).transpose(2, 1, 0, 3))
    wv4 = wv.reshape(D, KV, hd)[:, ksl]
    wvp = c(wv4.reshape(KO, 128, KVL * hd).transpose(1, 0, 2))
    wos = wo[g * HL * hd:(g + 1) * HL * hd]
    wop = c(wos.reshape(HL, hd, D).transpose(1, 0, 2))
    adp = c(adapter[0].T.reshape(KO, 128, A).transpose(1, 0, 2))
    # cos^T / sin^T, each duplicated across both partition halves
    ct = np.asarray(freqs_cos, dtype=f32).T      # [64, S]
    st = np.asarray(freqs_sin, dtype=f32).T
    csp = np.empty((128, 2, S), f32)
    csp[0:64, 0] = ct
    csp[64:128, 0] = ct
    csp[0:64, 1] = st
    csp[64:128, 1] = st
    tri = c(np.triu(np.ones((128, 128), dtype=f32)))
    gth = np.tanh(np.asarray(gate[0, hsl, 0, 0], dtype=np.float64)).astype(f32)
    gcp = c(np.repeat(gth, 128).reshape(1, HL * 128))
    inp = {"xp": xp, "wqp": wqp, "wkp": wkp, "wvp": wvp, "wop": wop,
           "adp": adp, "csp": csp, "trip": tri, "gcp": gcp}
    if not causal:
        mt = np.asarray(mask).reshape(S, S).T  # [keys, q]
        inp["mtp"] = c(mt.reshape(KC, 128, S).transpose(1, 0, 2), f32)
    return inp


# --------------------------------------------------------------------------
# entry point
# --------------------------------------------------------------------------

def kernel(x, wq, wk, wv, wo, adapter, gate, freqs_cos, freqs_sin, mask,
           _trace=False):
    x, wq, wk, wv, wo, adapter, gate, freqs_cos, freqs_sin, mask = (
        np.asarray(a) for a in
        (x, wq, wk, wv, wo, adapter, gate, freqs_cos, freqs_sin, mask))
    B, S, D = x.shape
    H = gate.shape[1]
    hd = wq.shape[1] // H
    KV = wk.shape[1] // hd
    G = 8 // B                      # head groups per batch over 8 cores
    HL, KVL = H // G, KV // G
    KO = D // 128

    causal = is_causal_mask(mask)
    nc = get_program(KO, S, HL, KVL, causal, MM_MODE)

    in_maps = [prep_core_inputs(core, G, x, wq, wk, wv, wo, adapter, gate,
                                freqs_cos, freqs_sin, mask, causal)
               for core in range(8)]
    res = run_bass_kernel_spmd(nc, in_maps, core_ids=list(range(8)),
                               trace=_trace)
    out = np.zeros((B, S, D), np.float32)
    for core in range(8):
        b = core // G
        r = res.results[core]
        out[b] += r["out"].reshape(S, D)
        if "out2" in r:
            out[b] += r["out2"].reshape(S, D)
    if _trace:
        kernel._last_result = res
    return out



# revision 68
# speedup vs baseline: 1.2442x; 1.2442x over previous
"""Trainium2 Bass kernel for nn_Attention_50216757625003.

GQA attention layer: B=2, S=1024, D=4096, H=32 q-heads, KV=8 kv-heads,
hd=128, A=10 gated adapter tokens, RoPE, split softmax (adapter block
softmaxed separately and scaled by tanh(gate)), causal mask.

Sharding (8 NeuronCores): outer data-parallel over batch (2) x
tensor-parallel over heads (4 groups of 8 q-heads / 2 kv-heads).
wq/wk/wv are sharded column-wise, wo row-wise; each core computes a
partial [S, D] output contribution and the host sums the 4 head-group
partials per batch element.

Pipeline structure (single PE instruction queue is in-order, so emission
order is the schedule):
  stage A: V projection + K projection streamed together over x-chunk
    groups (both ready early); adapter K/V ride the same weight stream.
  stage B: per-head software pipeline - Q projection of head h is
    interleaved at weight-block granularity with the attention of head
    h-1, so the Act-engine softmax chain never stalls the PE.
  stage C: output projection, streaming wo with the first block
    prefetched during stage B.

Cost-model-aware tricks:
  * matmul cost = output free size (contraction depth is free), so the
    softmax denominators are computed with ap_size=1 matmuls
    (probs^T @ ones -> [q,1] columns) instead of [1,q] ones-row matmuls
    that cost as much as the PV matmul itself.
  * the per-q normalization scales are assembled via one small PE
    transpose + GpSimd partition_broadcast (Pool engine is otherwise
    idle), freeing the PE of rank-1 broadcast matmuls.
  * tanh(gate) is folded into the adapter denominator matmul (rhs =
    1/tanh(g_h) instead of ones), so no extra gating multiply exists.
  * scores are built transposed ([keys, q]) so probs feed the PV matmul
    directly; softmax max-subtraction is skipped (scores are O(1)).
  * bf16 operands (KMM=bf16 default): same PE rate as f32r for wide
    matmuls but 1 cyc/row for narrow ones, and half the DMA traffic.
"""

import os
import sys

import numpy as np

for _p in ("/opt/trn_rl_repo",):
    if _p not in sys.path and os.path.isdir(_p):
        sys.path.insert(0, _p)

import concourse.bass as bass
import concourse.mybir as mybir
from concourse import bacc
import concourse.tile as tile
from concourse.bass_utils import run_bass_kernel_spmd

HD = 128  # head dim (hardcoded: rope split + tile shapes assume 128)
A = 10    # adapter tokens
F32 = mybir.dt.float32

MM_MODE = os.environ.get("KMM", "bf16")

_PROG_CACHE = {}


def _md(mm):
    return {"f32r": mybir.dt.float32r, "f32": mybir.dt.float32,
            "bf16": mybir.dt.bfloat16}[mm]


# --------------------------------------------------------------------------
# device program
# --------------------------------------------------------------------------

def build_program(KO, S, HL, KVL, causal, mm):
    """One NeuronCore's program.

    KO: D // 128 contraction chunks.  S: sequence length.  HL: q heads on
    this core.  KVL: kv heads on this core.  causal: hardwire causal
    masking (tri mask on diagonal chunks + chunk skipping); otherwise an
    additive mask [S, S] is an input.  mm: matmul operand dtype mode.
    """
    nc = bacc.Bacc(None, target_bir_lowering=False,
                   dynamic_dma_scratch_size=2048)
    MD = _md(mm)
    D = KO * 128
    QB = min(512, S)       # q column block
    NQH = S // QB
    NSUB = QB // 128       # q sub-blocks per block
    KC = S // 128          # token key chunks
    SA = S + A
    nrep = HL // KVL

    xp = nc.declare_dram_parameter("xp", [128, KO, S], MD, isOutput=False)
    wqp = nc.declare_dram_parameter("wqp", [HL, KO // 8, 128, 8 * HD], MD, isOutput=False)
    wkp = nc.declare_dram_parameter("wkp", [KVL, 128, KO, HD], MD, isOutput=False)
    wvp = nc.declare_dram_parameter("wvp", [128, KO, KVL * HD], MD, isOutput=False)
    wop = nc.declare_dram_parameter("wop", [128, HL, D], MD, isOutput=False)
    adp = nc.declare_dram_parameter("adp", [128, KO, A], MD, isOutput=False)
    csp = nc.declare_dram_parameter("csp", [128, 2, S], F32, isOutput=False)
    trip = nc.declare_dram_parameter("trip", [128, 128], MD, isOutput=False)
    idp = nc.declare_dram_parameter("idp", [128, 128], F32, isOutput=False)
    gdp = nc.declare_dram_parameter("gdp", [A, HL], MD, isOutput=False)
    if not causal:
        mtp = nc.declare_dram_parameter("mtp", [128, KC, S], F32, isOutput=False)
    outp = nc.declare_dram_parameter("out", [128, S // 128, D], F32, isOutput=True)
    KDBG = os.environ.get("KDBG")
    if KDBG:
        dbgp = nc.declare_dram_parameter("dbg", [128, 4096], F32, isOutput=True)

    Exp = mybir.ActivationFunctionType.Exp
    XG = min(4, KO)
    NX = KO // XG

    with tile.TileContext(nc) as tc:
        with tc.tile_pool(name="singles", bufs=1) as singles, \
             tc.tile_pool(name="persist", bufs=1) as persist:
            # resident x^T chunk-group tiles, DMA'd just-in-time
            xt = [persist.tile([128, XG, S], MD, tag=f"x{i}", name=f"x{i}")
                  for i in range(NX)]
            xt_loaded = [False] * NX

            def xload(i):
                if not xt_loaded[i]:
                    h = XG // 2 or 1
                    nc.sync.dma_start(xt[i][:, 0:h, :],
                                      xp[:, i * XG:i * XG + h, :])
                    if h < XG:
                        nc.sync.dma_start(xt[i][:, h:XG, :],
                                          xp[:, i * XG + h:(i + 1) * XG, :])
                    xt_loaded[i] = True

            def xsl(c):
                return xt[c // XG][:, c % XG, :]

            tri = singles.tile([128, 128], MD)
            adT = singles.tile([128, KO, A], MD)
            gdv = singles.tile([A, HL], MD)
            ident = singles.tile([128, 128], F32)
            csd = singles.tile([128, 2, S], F32)
            # g=0 weight tiles race ahead of the table DMAs so the first
            # matmul only waits for x chunk 0 + its weights
            wv0 = singles.tile([128, XG, KVL * HD], MD)
            wk0 = [singles.tile([128, XG, HD], MD, name=f"wk0_{j}")
                   for j in range(KVL)]
            # startup order: x chunk 0 + K g0 weights first (the first PE
            # work is kblock(0,0) paced chunk-by-chunk), then the rest
            nc.sync.dma_start(xt[0][:, 0:1, :], xp[:, 0:1, :])
            for j in range(KVL):
                nc.sync.dma_start(wk0[j], wkp[j, :, 0:XG, :])
            nc.sync.dma_start(xt[0][:, 1:XG, :], xp[:, 1:XG, :])
            xt_loaded[0] = True
            nc.sync.dma_start(adT, adp[:])
            nc.sync.dma_start(wv0, wvp[:, 0:XG, :])
            nc.sync.dma_start(tri, trip[:])
            nc.sync.dma_start(gdv, gdp[:])
            nc.sync.dma_start(ident, idp[:])
            nc.sync.dma_start(csd, csp[:])
            csA = csd[:, 0, :]
            csB = csd[:, 1, :]

            ones_col = tri[:, 127:128]   # all-ones [128,1] (MD)
            zb = singles.tile([128, 1], F32)
            nc.vector.memset(zb, 0.0)

            kT = [persist.tile([128, SA], MD, tag=f"kT{j}", name=f"kT{j}")
                  for j in range(KVL)]
            vv = persist.tile([128, KC + 1, KVL * HD], MD, tag="vv")
            qT = [persist.tile([128, S], MD, tag=f"qT{h}", name=f"qT{h}")
                  for h in range(HL)]
            oTt = [persist.tile([128, 4, S], MD, tag=f"oT{i}", name=f"oT{i}")
                   for i in range((HL + 3) // 4)]

            def oT(h):
                return oTt[h // 4][:, h % 4, :]

            # wo prefetch tiles for the first two n-blocks of stage C
            NB = D // 512
            wopre = [persist.tile([128, HL, 512], MD, tag=f"wopre{i}",
                                  name=f"wopre{i}") for i in range(2)]
            # head-0 Q weight prefetch (DMA'd late in stage A so stage B
            # starts without a weight stall)
            wq0pre = [persist.tile([128, 8, HD], MD, tag=f"wq0pre{i}",
                                   name=f"wq0pre{i}") for i in range(2)]

            if not causal:
                mtt = persist.tile([128, KC, S], F32, tag="mt")
                nc.sync.dma_start(mtt[:, 0:KC // 2, :], mtp[:, 0:KC // 2, :])
                nc.sync.dma_start(mtt[:, KC // 2:KC, :], mtp[:, KC // 2:KC, :])

                def mtsl(kc):
                    return mtt[:, kc, :]

            with tc.tile_pool(name="rpool", bufs=2) as rpool, \
                 tc.tile_pool(name="cpool", bufs=1) as cpool:

                def rope_copy(ps_h):
                    rc = rpool.tile([128, QB], F32, tag="rc", bufs=4)
                    nc.vector.tensor_copy(rc, ps_h)  # frees the psum fast;
                    # DVE, so the Act exp queue stays short
                    return rc

                def rope_rest(rc, dst, hh):
                    # rc rows 0:64 = x0 (even pair elems), 64:128 = x1.
                    # dst[0:64] = x0*cos - x1*sin ; dst[64:128] = x0*sin + x1*cos
                    sl = slice(hh * QB, (hh + 1) * QB)
                    rs = rpool.tile([128, QB], F32, tag="rs", bufs=2)
                    nc.sync.dma_start(rs[0:64, :], rc[64:128, :])
                    nc.sync.dma_start(rs[64:128, :], rc[0:64, :])
                    tm1 = rpool.tile([128, QB], F32, tag="tm1", bufs=1)
                    tm2 = rpool.tile([128, QB], F32, tag="tm2", bufs=1)
                    nc.vector.tensor_mul(tm1, rc, csA[:, sl])
                    nc.vector.tensor_mul(tm2, rs, csB[:, sl])
                    nc.vector.tensor_sub(dst[0:64, sl], tm1[0:64, :], tm2[0:64, :])
                    nc.vector.tensor_add(dst[64:128, sl], tm2[64:128, :],
                                         tm1[64:128, :])

                def emit_rope(ps_h, dst, hh):
                    rope_rest(rope_copy(ps_h), dst, hh)

                # ============ stage A: V + K projections ==================
                with tc.tile_pool(name="wpoolA", bufs=3) as wpoolA, \
                     tc.tile_pool(name="psA", bufs=1, space="PSUM") as psA:
                    vacc = cpool.tile([128, KC, KVL * HD], F32)
                    krc = [[None] * NQH for _ in range(KVL)]
                    psk = [[psA.tile([128, QB], F32, tag=f"psk{j}_{hh}",
                                     name=f"psk{j}_{hh}")
                            for hh in range(NQH)] for j in range(KVL)]
                    pav = psA.tile([A, KVL * HD], F32, tag="pav")
                    pakk = psA.tile([128, KVL * A], F32, tag="pakk")
                    NVB = KO // XG   # V/K stream in x-group-sized blocks

                    def emit_vblock(g):
                        if g == 0:
                            wt = wv0
                        else:
                            wt = wpoolA.tile([128, XG, KVL * HD], MD, tag="wv")
                            nc.sync.dma_start(wt, wvp[:, g * XG:(g + 1) * XG, :])
                        xload(g)
                        for t in range(KC):
                            psv = psA.tile([128, KVL * HD], F32, tag="vproj",
                                           bufs=2)
                            for ci in range(XG):
                                nc.tensor.matmul(
                                    psv[:, :],
                                    xsl(g * XG + ci)[:, t * 128:(t + 1) * 128],
                                    wt[:, ci, :],
                                    start=(ci == 0), stop=(ci == XG - 1))
                            if g == 0 and NVB > 1:
                                nc.scalar.copy(vacc[:, t, :], psv[:, :])
                            elif g < NVB - 1:
                                nc.vector.tensor_add(vacc[:, t, :],
                                                     vacc[:, t, :], psv[:, :])
                            elif NVB > 1:
                                nc.vector.tensor_add(vv[:, t, :],
                                                     vacc[:, t, :], psv[:, :])
                            else:
                                nc.scalar.copy(vv[:, t, :], psv[:, :])
                        for ci in range(XG):
                            c = g * XG + ci
                            nc.tensor.matmul(pav[:, :], adT[:, c, :],
                                             wt[:, ci, :],
                                             start=(c == 0), stop=(c == KO - 1))
                        if g == NVB - 1:
                            nc.scalar.copy(vv[0:A, KC, :], pav[:, :])

                    def emit_kblock(j, g):
                        if g == 0:
                            wt = wk0[j]
                        else:
                            wt = wpoolA.tile([128, XG, HD], MD, tag="wk")
                            nc.sync.dma_start(wt,
                                              wkp[j, :, g * XG:(g + 1) * XG, :])
                        for ci in range(XG):
                            c = g * XG + ci
                            for hh in range(NQH):
                                sl = slice(hh * QB, (hh + 1) * QB)
                                nc.tensor.matmul(
                                    psk[j][hh][:, :], wt[:, ci, :],
                                    xsl(c)[:, sl],
                                    start=(c == 0), stop=(c == KO - 1))
                        for ci in range(XG):
                            c = g * XG + ci
                            # psum start=True poisons the whole 2KB zero
                            # region (bank row): only the first matmul into
                            # the pakk bank may set it; later first-touches
                            # overwrite via the pending-zero bytes
                            nc.tensor.matmul(
                                pakk[:, j * A:(j + 1) * A], wt[:, ci, :],
                                adT[:, c, :],
                                start=(j == 0 and c == 0),
                                stop=(j == KVL - 1 and c == KO - 1),
                                skip_group_check=True)
                        if g == NX - 1:
                            # copy psum out now (frees psk for stage B);
                            # the rope tails are emitted after the psA pool
                            # closes so its exit barrier doesn't chain
                            # stage B behind the whole rope DVE/DMA chain
                            for hh in range(NQH):
                                krc[j][hh] = rope_copy(psk[j][hh])
                            nc.scalar.copy(kT[j][:, S:SA],
                                           pakk[:, j * A:(j + 1) * A])

                    for g in range(NX):
                        if g == 0:
                            # first group: K is paced chunk-by-chunk by the
                            # x DMA (V needs all 4 chunks at once)
                            xload(g)
                            for j in range(KVL):
                                emit_kblock(j, g)
                            emit_vblock(g)
                        else:
                            # V first everywhere else; in the last group its
                            # DVE add-drain then overlaps the K blocks (the
                            # psA pool close waits on all of it)
                            emit_vblock(g)
                            for j in range(KVL):
                                emit_kblock(j, g)
                        if g == NX - 3:
                            for i in range(2):
                                nc.sync.dma_start(wq0pre[i], wqp[0, i])

                # K rope tails (outside psA so its exit barrier is cheap)
                for j in range(KVL):
                    for hh in range(NQH):
                        rope_rest(krc[j][hh], kT[j], hh)

                # ============ stage B: Q projections + attention ==========
                spool_cm = tc.tile_pool(name="spool", bufs=3)
                spool = spool_cm.__enter__()
                wpoolB_cm = tc.tile_pool(name="wpoolB", bufs=3)
                wpoolB = wpoolB_cm.__enter__()
                ps2_cm = tc.tile_pool(name="ps2", bufs=1, space="PSUM")
                ps2 = ps2_cm.__enter__()
                pscur = [ps2]   # attention psum pool (swapped for stage C)
                if True:
                    def emit_qblock(h, b, psq, cis=range(8), hhs=None):
                        if h == 0 and b < 2:
                            wt = wq0pre[b]     # prefetched in stage A
                        elif psq[2] is not None:
                            wt = psq[2]
                        else:
                            wt = wpoolB.tile([128, 8, HD], MD, tag="wq")
                            nc.sync.dma_start(wt, wqp[h, b])
                        psq[2] = wt if cis[-1] != 7 else None
                        for ci in cis:
                            c = b * 8 + ci
                            st, sp = (c == 0), (c == KO - 1)
                            for hh in (range(NQH) if hhs is None else hhs):
                                sl = slice(hh * QB, (hh + 1) * QB)
                                nc.tensor.matmul(
                                    psq[hh][:, :], wt[:, ci, :], xsl(c)[:, sl],
                                    start=st, stop=sp)

                    def attn_open_a(h, qh):
                        """Allocate psum, emit chunk 0 + the adapter scores.
                        The adapter exp queues right behind chunk 0's; the
                        pa-dependent matmuls wait until attn_open_b (the
                        caller interleaves a qblock in between)."""
                        ps = pscur[0]
                        j = h // nrep
                        qs, qe = qh * QB, (qh + 1) * QB
                        if causal:
                            kcs = [kc for kc in range(KC) if kc * 128 < qe]
                        else:
                            kcs = list(range(KC))
                        ot = ps.tile([128, QB], F32, tag="ot", bufs=1)
                        oa = ps.tile([128, QB], F32, tag="oa", bufs=1)
                        # dn ([:, 0:8]) and the transposed reciprocals
                        # rT ([0:8, 8:136]) share one psum bank; the
                        # transpose runs only after the recip has read dn
                        dnrt = ps.tile([128, 2 * NSUB + 128], F32, tag="dn",
                                       bufs=1)
                        st = {"h": h, "qh": qh, "j": j, "qs": qs, "qe": qe,
                              "kcs": kcs, "ot": ot, "oa": oa, "dnrt": dnrt,
                              "ki": 0, "ps": ps}
                        attn_chunk(st)
                        sca = ps.tile([128, QB], F32, tag="scp", bufs=3)
                        nc.tensor.matmul(sca[0:A, :], kT[j][:, S:SA],
                                         qT[h][:, qs:qe], start=True, stop=True)
                        pa = spool.tile([128, QB], MD, tag="pt", bufs=6)
                        nc.scalar.activation(pa[0:A, :], sca[0:A, :], Exp,
                                             bias=zb[0:A, :])
                        st["pa"] = pa
                        return st

                    def attn_open_b(st):
                        """Adapter PV + denominators (pa exp has landed)."""
                        h, j, oa, pa = st["h"], st["j"], st["oa"], st["pa"]
                        nc.tensor.matmul(oa[:, :],
                                         vv[0:A, KC, j * HD:(j + 1) * HD],
                                         pa[0:A, :], start=True, stop=True)
                        dn = st["dnrt"][:, 0:2 * NSUB]
                        # start=False: these land while the bank is pending-
                        # zero from chunk 0's opener, so first touch
                        # overwrites (zero-region semantics)
                        for s in range(NSUB):
                            nc.tensor.matmul(
                                dn[:, NSUB + s:NSUB + s + 1],
                                pa[0:A, s * 128:(s + 1) * 128],
                                gdv[:, h:h + 1], start=False, stop=False,
                                skip_group_check=True)
                        oasb = spool.tile([128, QB], F32, tag="oasb", bufs=2)
                        nc.vector.tensor_copy(oasb, oa)
                        st["oasb"] = oasb

                    def attn_open(h, qh):
                        st = attn_open_a(h, qh)
                        attn_open_b(st)
                        return st

                    def attn_chunk(st, n=1):
                        """Emit the next n score/exp/PV/denominator chunks.
                        After the last chunk, spill ot to SBUF so the psum
                        bank frees without waiting for the norm chain."""
                        h, qh, j = st["h"], st["qh"], st["j"]
                        qs, qe, kcs = st["qs"], st["qe"], st["kcs"]
                        ot, dnrt, ps = st["ot"], st["dnrt"], st["ps"]
                        dn = dnrt[:, 0:2 * NSUB]
                        for _ in range(n):
                            ki = st["ki"]
                            kc = kcs[ki]
                            st["ki"] = ki + 1
                            q0 = max(qs, kc * 128) if causal else qs
                            N = qe - q0
                            scp = ps.tile([128, QB], F32, tag="scp", bufs=3)
                            nc.tensor.matmul(
                                scp[:, 0:N],
                                kT[j][:, kc * 128:(kc + 1) * 128],
                                qT[h][:, q0:qe], start=True, stop=True)
                            pt = spool.tile([128, QB], MD, tag="pt", bufs=6)
                            if causal:
                                nc.scalar.activation(pt[:, 0:N], scp[:, 0:N],
                                                     Exp, bias=zb)
                                if kc * 128 >= qs:  # diagonal chunk
                                    nc.vector.tensor_mul(
                                        pt[:, 0:128], pt[:, 0:128], tri)
                            else:
                                sadd = spool.tile([128, QB], F32, tag="sadd",
                                                  bufs=2)
                                nc.vector.tensor_add(
                                    sadd[:, 0:N], scp[:, 0:N],
                                    mtsl(kc)[:, q0:qe])
                                nc.scalar.activation(pt[:, 0:N], sadd[:, 0:N],
                                                     Exp, bias=zb)
                            nc.tensor.matmul(
                                ot[:, q0 - qs:QB],
                                vv[:, kc, j * HD:(j + 1) * HD],
                                pt[:, 0:N], start=(ki == 0),
                                stop=(ki == len(kcs) - 1))
                            # denominators: probs^T @ ones -> [q,1] columns.
                            # Only the very first matmul into the dn bank
                            # sets start (zero-region semantics); later
                            # columns' first touches overwrite via the
                            # pending-zero bytes.
                            for s in range(NSUB):
                                qa = qs + s * 128
                                if qa < q0:
                                    continue
                                off = qa - q0
                                nc.tensor.matmul(
                                    dn[:, s:s + 1], pt[:, off:off + 128],
                                    ones_col,
                                    start=(ki == 0 and s == 0),
                                    stop=(ki == len(kcs) - 1
                                          and s == NSUB - 1),
                                    skip_group_check=True)
                        if st["ki"] == len(kcs):
                            otsb = spool.tile([128, QB], F32, tag="otsb",
                                              bufs=2)
                            nc.scalar.copy(otsb, ot)
                            st["otsb"] = otsb

                    def attn_norm(st):
                        """reciprocal -> per-column transposes -> broadcast
                        -> scale+sum.  Each [128,1] reciprocal column is
                        PE-transposed to a [1,128] row at partition 0 (Pool
                        partition_broadcast requires partition-0 sources),
                        all landing in one [1, 8*128] psum row."""
                        h, qh, dnrt = st["h"], st["qh"], st["dnrt"]
                        qs, qe = st["qs"], st["qe"]
                        ps = st["ps"]
                        rcp = spool.tile([128, 2 * NSUB], F32, tag="rcp",
                                         bufs=2)
                        nc.vector.reciprocal(rcp, dnrt[:, 0:2 * NSUB])
                        # one f32 transpose into the dn bank (runs after the
                        # recip read), then DVE -> SBUF, then a tiny
                        # SBUF->SBUF DMA flattens [8,128] onto partition 0
                        # so the Pool broadcasts have legal sources
                        rT = dnrt[0:2 * NSUB, 2 * NSUB:2 * NSUB + 128]
                        nc.tensor.matmul(rT, rcp, ident, is_transpose=True,
                                         skip_group_check=True)
                        rTs = spool.tile([2 * NSUB, 128], F32, tag="rTs",
                                         bufs=2)
                        nc.vector.tensor_copy(rTs, rT)
                        rfs = spool.tile([1, 2 * NSUB * 128], F32, tag="rfs",
                                         bufs=2)
                        hw = NSUB * 128
                        nc.sync.dma_start(rfs[0:1, 0:hw], rTs[0:NSUB, :])
                        nc.sync.dma_start(rfs[0:1, hw:2 * hw], rTs[NSUB:, :])
                        rtb = spool.tile([128, QB], F32, tag="rtb", bufs=2)
                        rab = spool.tile([128, QB], F32, tag="rab", bufs=2)
                        for s in range(NSUB):
                            nc.gpsimd.partition_broadcast(
                                rtb[:, s * 128:(s + 1) * 128],
                                rfs[0:1, s * 128:(s + 1) * 128])
                            nc.gpsimd.partition_broadcast(
                                rab[:, s * 128:(s + 1) * 128],
                                rfs[0:1, (NSUB + s) * 128:(NSUB + s + 1) * 128])
                        # tq2 + the final add run on Pool (idle) so the DVE
                        # queue (rope + spills) never gates the oT writes
                        tq1 = spool.tile([128, QB], F32, tag="tq1", bufs=1)
                        nc.vector.tensor_mul(tq1, st["otsb"], rtb)
                        tq2 = spool.tile([128, QB], F32, tag="tq2", bufs=1)
                        nc.gpsimd.tensor_mul(tq2, st["oasb"], rab)
                        nc.gpsimd.tensor_add(oT(h)[:, qs:qe], tq1, tq2)
                        if KDBG and h == 0 and qh == 0:
                            dcp = spool.tile([128, 8], F32, tag="dcp", bufs=1)
                            nc.vector.tensor_copy(dcp, dnrt)
                            nc.sync.dma_start(dbgp[:, 0:8], dcp)
                            rfc = spool.tile([1, 1024], F32, tag="rfc", bufs=1)
                            nc.vector.tensor_copy(rfc, rfs)
                            nc.sync.dma_start(dbgp[0:1, 1024:2048], rfc)
                            rtc = spool.tile([128, QB], F32, tag="rtc", bufs=1)
                            nc.vector.tensor_copy(rtc, rtb)
                            nc.sync.dma_start(dbgp[:, 2048:2560], rtc)
                            nc.sync.dma_start(dbgp[:, 2560:3072], st["otsb"])
                            nc.sync.dma_start(dbgp[:, 3072:3584], tq1)

                    # last-head weight tiles, prefetched a head early
                    wtN = [wpoolB.tile([128, 8, HD], MD, tag=f"wqN{b}",
                                       bufs=1, name=f"wqN{b}")
                           for b in range(4)]

                    # per-head software pipeline: project h, attend h-1
                    for h in range(HL - 1):
                        if h == HL - 2:
                            for b in range(4):
                                nc.sync.dma_start(wtN[b], wqp[HL - 1, b])
                        if h == 2:
                            # prefetch wo for stage C's first two n-blocks
                            # (DMA is idle here; stage C is far away)
                            for i in range(2):
                                for wpi in range(0, HL, 4):
                                    nc.sync.dma_start(
                                        wopre[i][:, wpi:wpi + 4, :],
                                        wop[:, wpi:wpi + 4,
                                            i * 512:(i + 1) * 512])
                        psq = [ps2.tile([128, QB], F32, tag=f"psq{hh}",
                                        bufs=1, name=f"psq{h}_{hh}")
                               for hh in range(NQH)] + [None]
                        emit_qblock(h, 0, psq)
                        emit_qblock(h, 1, psq)
                        if h > 0:
                            st0 = attn_open_a(h - 1, 0)
                        emit_qblock(h, 2, psq, range(4))
                        if h > 0:
                            attn_open_b(st0)
                            attn_chunk(st0, len(st0["kcs"]) - 1)
                        emit_qblock(h, 2, psq, range(4, 8))
                        if h > 0:
                            attn_norm(st0)
                            st1 = attn_open_a(h - 1, 1)
                        emit_qblock(h, 3, psq, range(4))
                        if h > 0:
                            attn_open_b(st1)
                            attn_chunk(st1, len(st1["kcs"]) - 1)
                        emit_qblock(h, 3, psq, range(4, 8))
                        if h > 0:
                            attn_norm(st1)
                        for hh in range(NQH):
                            emit_rope(psq[hh], qT[h], hh)

                    # last head hN: project hh-halves separately so its rope
                    # lands early; its qh=0 attention runs here in stage B,
                    # hidden under the hh=1 projection pass
                    hN = HL - 1
                    psq = [ps2.tile([128, QB], F32, tag=f"psq{hh}",
                                    bufs=1, name=f"psqN_{hh}")
                           for hh in range(NQH)] + [None]

                    def qblockN(b, hh):
                        for ci in range(8):
                            c = b * 8 + ci
                            nc.tensor.matmul(
                                psq[hh][:, :], wtN[b][:, ci, :],
                                xsl(c)[:, hh * QB:(hh + 1) * QB],
                                start=(c == 0), stop=(c == KO - 1))

                    qblockN(0, 0)
                    st0 = attn_open_a(hN - 1, 0)
                    qblockN(1, 0)
                    attn_open_b(st0)
                    attn_chunk(st0, len(st0["kcs"]) - 1)
                    qblockN(2, 0)
                    attn_norm(st0)
                    st1 = attn_open_a(hN - 1, 1)
                    qblockN(3, 0)
                    attn_open_b(st1)
                    attn_chunk(st1, 3)
                    attn_chunk(st1, len(st1["kcs"]) - 4)
                    qblockN(0, 1)
                    attn_norm(st1)
                    emit_rope(psq[0], qT[hN], 0)
                    qblockN(1, 1)
                    stN0 = attn_open_a(hN, 0)
                    qblockN(2, 1)
                    attn_open_b(stN0)
                    attn_chunk(stN0, len(stN0["kcs"]) - 1)
                    qblockN(3, 1)
                    attn_norm(stN0)
                    emit_rope(psq[1], qT[hN], 1)

                # close stage-B psum pool so stage C can reuse its banks
                ps2_cm.__exit__(None, None, None)
                wpoolB_cm.__exit__(None, None, None)

                # ===== stage C: last head's attention + output projection ==
                with tc.tile_pool(name="wopool", bufs=2) as wopool, \
                     tc.tile_pool(name="obpool", bufs=2) as obpool, \
                     tc.tile_pool(name="ps3", bufs=1, space="PSUM") as ps3:
                    pscur[0] = ps3
                    MB = S // 128

                    ob_cur = [None]

                    def emit_wo(n, won, ms, single_dma=False):
                        for m in ms:
                            pso = ps3.tile([128, 512], F32, tag="wo", bufs=2)
                            for hh2 in range(HL):
                                nc.tensor.matmul(
                                    pso,
                                    oT(hh2)[:, m * 128:(m + 1) * 128],
                                    won[:, hh2, :],
                                    start=(hh2 == 0), stop=(hh2 == HL - 1))
                            if single_dma:
                                ob = obpool.tile([128, 2, 512], F32,
                                                 tag="ob", name=f"ob{n}_{m}")
                                nc.scalar.copy(ob[:, 0, :], pso)
                                nc.sync.dma_start(
                                    outp[:, m:m + 1,
                                         n * 512:(n + 1) * 512],
                                    ob[:, 0:1, :])
                                continue
                            if m % 2 == 0:
                                ob_cur[0] = obpool.tile([128, 2, 512], F32,
                                                        tag="ob",
                                                        name=f"ob{n}_{m}")
                            ob = ob_cur[0]
                            nc.scalar.copy(ob[:, m % 2, :], pso)
                            if m % 2 == 1:
                                nc.sync.dma_start(
                                    outp[:, m - 1:m + 1,
                                         n * 512:(n + 1) * 512], ob)

                    # interleave the last attention block (hN, qh=1) with
                    # the first WO n-block: m0-3 only need oT(hN) qh=0,
                    # which stage B already produced
                    emit_wo(0, wopre[0], range(0, 1))
                    stc1 = attn_open_a(hN, 1)
                    emit_wo(0, wopre[0], range(1, 2))
                    attn_open_b(stc1)
                    attn_chunk(stc1, 3)
                    emit_wo(0, wopre[0], range(2, 3))
                    attn_chunk(stc1, len(stc1["kcs"]) - 4)
                    emit_wo(0, wopre[0], range(3, 4))
                    attn_norm(stc1)
                    # n=1's m0-3 only need the qh=0 norms: they run while
                    # the last norm chain (Pool/DVE) finishes
                    emit_wo(1, wopre[1], range(0, 4))
                    emit_wo(0, wopre[0], range(4, MB))
                    emit_wo(1, wopre[1], range(4, MB))
                    for n in range(2, NB):
                        won = wopool.tile([128, HL, 512], MD, tag="won")
                        for wpi in range(0, HL, 4):
                            nc.sync.dma_start(
                                won[:, wpi:wpi + 4, :],
                                wop[:, wpi:wpi + 4,
                                    n * 512:(n + 1) * 512])
                        emit_wo(n, won, range(MB))

                spool_cm.__exit__(None, None, None)

    nc.compile()
    nc.finalize()
    return nc


def get_program(KO, S, HL, KVL, causal, mm):
    key = (KO, S, HL, KVL, causal, mm)
    if key not in _PROG_CACHE:
        _PROG_CACHE[key] = build_program(KO, S, HL, KVL, causal, mm)
    return _PROG_CACHE[key]


# --------------------------------------------------------------------------
# host-side sharding / layout prep
# --------------------------------------------------------------------------

_EVEN_FIRST = np.concatenate([np.arange(0, HD, 2), np.arange(1, HD, 2)])


def is_causal_mask(mask):
    S = mask.shape[-1]
    m = np.asarray(mask).reshape(S, S)
    iu = np.triu_indices(S, 1)
    il = np.tril_indices(S)
    return bool(np.all(m[il] == 0.0) and np.all(m[iu] <= -1e8))


def _np_md(mm):
    if mm == "bf16":
        import ml_dtypes
        return ml_dtypes.bfloat16
    return np.float32


def prep_core_inputs(core, G, x, wq, wk, wv, wo, adapter, gate,
                     freqs_cos, freqs_sin, mask, causal, mm=None):
    """Build the input dict for one core = (batch b, head-group g)."""
    mm = MM_MODE if mm is None else mm
    B, S, D = x.shape
    H = gate.shape[1]
    hd = wq.shape[1] // H
    KV = wk.shape[1] // hd
    KO = D // 128
    KC = S // 128
    HL, KVL = H // G, KV // G
    b, g = core // G, core % G
    hsl = slice(g * HL, (g + 1) * HL)
    ksl = slice(g * KVL, (g + 1) * KVL)
    idx = _EVEN_FIRST
    f32 = np.float32
    md = _np_md(mm)

    def c(a, dt=None):
        return np.ascontiguousarray(a, dtype=dt if dt is not None else md)

    xp = c(x[b].T.reshape(KO, 128, S).transpose(1, 0, 2))
    wq4 = wq.reshape(D, H, hd)[:, hsl][:, :, idx] * np.float32(1.0 / np.sqrt(hd))
    wqp = c(wq4.reshape(KO, 128, HL, hd).transpose(2, 1, 0, 3)
            .reshape(HL, 128, KO // 8, 8 * hd).transpose(0, 2, 1, 3))
    wk4 = wk.reshape(D, KV, hd)[:, ksl][:, :, idx]
    wkp = c(wk4.reshape(KO, 128, KVL, hd).transpose(2, 1, 0, 3))
    wv4 = wv.reshape(D, KV, hd)[:, ksl]
    wvp = c(wv4.reshape(KO, 128, KVL * hd).transpose(1, 0, 2))
    wos = wo[g * HL * hd:(g + 1) * HL * hd]
    wop = c(wos.reshape(HL, hd, D).transpose(1, 0, 2))
    adp = c(adapter[0].T.reshape(KO, 128, A).transpose(1, 0, 2))
    # cos^T / sin^T, each duplicated across both partition halves
    ct = np.asarray(freqs_cos, dtype=f32).T      # [64, S]
    st = np.asarray(freqs_sin, dtype=f32).T
    csp = np.empty((128, 2, S), f32)
    csp[0:64, 0] = ct
    csp[64:128, 0] = ct
    csp[0:64, 1] = st
    csp[64:128, 1] = st
    tri = c(np.triu(np.ones((128, 128), dtype=f32)))
    idp = np.eye(128, dtype=f32)
    gth = np.tanh(np.asarray(gate[0, hsl, 0, 0], dtype=np.float64))
    gdp = c(np.broadcast_to((1.0 / gth).astype(f32), (A, HL)))
    inp = {"xp": xp, "wqp": wqp, "wkp": wkp, "wvp": wvp, "wop": wop,
           "adp": adp, "csp": csp, "trip": tri, "idp": idp, "gdp": gdp}
    if not causal:
        mt = np.asarray(mask).reshape(S, S).T  # [keys, q]
        inp["mtp"] = c(mt.reshape(KC, 128, S).transpose(1, 0, 2), f32)
    return inp


# --------------------------------------------------------------------------
# entry point
# --------------------------------------------------------------------------

def kernel(x, wq, wk, wv, wo, adapter, gate, freqs_cos, freqs_sin, mask,
           _trace=False):
    x, wq, wk, wv, wo, adapter, gate, freqs_cos, freqs_sin, mask = (
        np.asarray(a) for a in
        (x, wq, wk, wv, wo, adapter, gate, freqs_cos, freqs_sin, mask))
    B, S, D = x.shape
    H = gate.shape[1]
    hd = wq.shape[1] // H
    KV = wk.shape[1] // hd
    G = 8 // B                      # head groups per batch over 8 cores
    HL, KVL = H // G, KV // G
    KO = D // 128

    causal = is_causal_mask(mask)
    nc = get_program(KO, S, HL, KVL, causal, MM_MODE)

    in_maps = [prep_core_inputs(core, G, x, wq, wk, wv, wo, adapter, gate,
                                freqs_cos, freqs_sin, mask, causal)
               for core in range(8)]
    res = run_bass_kernel_spmd(nc, in_maps, core_ids=list(range(8)),
                               trace=_trace)
    out = np.zeros((B, S, D), np.float32)
    for core in range(8):
        b = core // G
        r = res.results[core]
        # device layout: [128, S//128, D] partition-major
        out[b] += r["out"].transpose(1, 0, 2).reshape(S, D)
    if _trace:
        kernel._last_result = res
    return out


# revision 73
# speedup vs baseline: 1.2476x; 1.0027x over previous
"""Trainium2 Bass kernel for nn_Attention_50216757625003.

GQA attention layer: B=2, S=1024, D=4096, H=32 q-heads, KV=8 kv-heads,
hd=128, A=10 gated adapter tokens, RoPE, split softmax (adapter block
softmaxed separately and scaled by tanh(gate)), causal mask.

Sharding (8 NeuronCores): outer data-parallel over batch (2) x
tensor-parallel over heads (4 groups of 8 q-heads / 2 kv-heads).
wq/wk/wv are sharded column-wise, wo row-wise; each core computes a
partial [S, D] output contribution and the host sums the 4 head-group
partials per batch element.

Pipeline structure (single PE instruction queue is in-order, so emission
order is the schedule):
  stage A: V projection + K projection streamed together over x-chunk
    groups (both ready early); adapter K/V ride the same weight stream.
  stage B: per-head software pipeline - Q projection of head h is
    interleaved at weight-block granularity with the attention of head
    h-1, so the Act-engine softmax chain never stalls the PE.
  stage C: output projection, streaming wo with the first block
    prefetched during stage B.

Cost-model-aware tricks:
  * matmul cost = output free size (contraction depth is free), so the
    softmax denominators are computed with ap_size=1 matmuls
    (probs^T @ ones -> [q,1] columns) instead of [1,q] ones-row matmuls
    that cost as much as the PV matmul itself.
  * the per-q normalization scales are assembled via one small PE
    transpose + GpSimd partition_broadcast (Pool engine is otherwise
    idle), freeing the PE of rank-1 broadcast matmuls.
  * tanh(gate) is folded into the adapter denominator matmul (rhs =
    1/tanh(g_h) instead of ones), so no extra gating multiply exists.
  * scores are built transposed ([keys, q]) so probs feed the PV matmul
    directly; softmax max-subtraction is skipped (scores are O(1)).
  * bf16 operands (KMM=bf16 default): same PE rate as f32r for wide
    matmuls but 1 cyc/row for narrow ones, and half the DMA traffic.
"""

import os
import sys

import numpy as np

for _p in ("/opt/trn_rl_repo",):
    if _p not in sys.path and os.path.isdir(_p):
        sys.path.insert(0, _p)

import concourse.bass as bass
import concourse.mybir as mybir
from concourse import bacc
import concourse.tile as tile
from concourse.bass_utils import run_bass_kernel_spmd

HD = 128  # head dim (hardcoded: rope split + tile shapes assume 128)
A = 10    # adapter tokens
F32 = mybir.dt.float32

MM_MODE = os.environ.get("KMM", "bf16")

_PROG_CACHE = {}


def _md(mm):
    return {"f32r": mybir.dt.float32r, "f32": mybir.dt.float32,
            "bf16": mybir.dt.bfloat16}[mm]


# --------------------------------------------------------------------------
# device program
# --------------------------------------------------------------------------

def build_program(KO, S, HL, KVL, causal, mm):
    """One NeuronCore's program.

    KO: D // 128 contraction chunks.  S: sequence length.  HL: q heads on
    this core.  KVL: kv heads on this core.  causal: hardwire causal
    masking (tri mask on diagonal chunks + chunk skipping); otherwise an
    additive mask [S, S] is an input.  mm: matmul operand dtype mode.
    """
    nc = bacc.Bacc(None, target_bir_lowering=False,
                   dynamic_dma_scratch_size=2048)
    MD = _md(mm)
    D = KO * 128
    QB = min(512, S)       # q column block
    NQH = S // QB
    NSUB = QB // 128       # q sub-blocks per block
    KC = S // 128          # token key chunks
    SA = S + A
    nrep = HL // KVL

    xp = nc.declare_dram_parameter("xp", [128, KO, S], MD, isOutput=False)
    wqp = nc.declare_dram_parameter("wqp", [HL, KO // 8, 128, 8 * HD], MD, isOutput=False)
    wkp = nc.declare_dram_parameter("wkp", [KVL, 128, KO, HD], MD, isOutput=False)
    wvp = nc.declare_dram_parameter("wvp", [128, KO, KVL * HD], MD, isOutput=False)
    wop = nc.declare_dram_parameter("wop", [128, HL, D], MD, isOutput=False)
    adp = nc.declare_dram_parameter("adp", [128, KO, A], MD, isOutput=False)
    csp = nc.declare_dram_parameter("csp", [128, 2, S], F32, isOutput=False)
    trip = nc.declare_dram_parameter("trip", [128, 128], MD, isOutput=False)
    idp = nc.declare_dram_parameter("idp", [128, 128], F32, isOutput=False)
    gdp = nc.declare_dram_parameter("gdp", [A, HL], MD, isOutput=False)
    if not causal:
        mtp = nc.declare_dram_parameter("mtp", [128, KC, S], F32, isOutput=False)
    outp = nc.declare_dram_parameter("out", [128, S // 128, D], MD, isOutput=True)
    KDBG = os.environ.get("KDBG")
    if KDBG:
        dbgp = nc.declare_dram_parameter("dbg", [128, 4096], F32, isOutput=True)

    Exp = mybir.ActivationFunctionType.Exp
    XG = min(4, KO)
    NX = KO // XG

    with tile.TileContext(nc) as tc:
        with tc.tile_pool(name="singles", bufs=1) as singles, \
             tc.tile_pool(name="persist", bufs=1) as persist:
            # resident x^T chunk-group tiles, DMA'd just-in-time
            xt = [persist.tile([128, XG, S], MD, tag=f"x{i}", name=f"x{i}")
                  for i in range(NX)]
            xt_loaded = [False] * NX

            def xload(i):
                if not xt_loaded[i]:
                    h = XG // 2 or 1
                    nc.sync.dma_start(xt[i][:, 0:h, :],
                                      xp[:, i * XG:i * XG + h, :])
                    if h < XG:
                        nc.sync.dma_start(xt[i][:, h:XG, :],
                                          xp[:, i * XG + h:(i + 1) * XG, :])
                    xt_loaded[i] = True

            def xsl(c):
                return xt[c // XG][:, c % XG, :]

            tri = singles.tile([128, 128], MD)
            adT = singles.tile([128, KO, A], MD)
            gdv = singles.tile([A, HL], MD)
            ident = singles.tile([128, 128], F32)
            csd = singles.tile([128, 2, S], F32)
            # g=0 weight tiles race ahead of the table DMAs so the first
            # matmul only waits for x chunk 0 + its weights
            wv0 = singles.tile([128, XG, KVL * HD], MD)
            wk0 = [singles.tile([128, XG, HD], MD, name=f"wk0_{j}")
                   for j in range(KVL)]
            # startup order: x chunk 0 + K g0 weights first (the first PE
            # work is kblock(0,0) paced chunk-by-chunk), then the rest
            nc.sync.dma_start(xt[0][:, 0:1, :], xp[:, 0:1, :])
            for j in range(KVL):
                nc.sync.dma_start(wk0[j], wkp[j, :, 0:XG, :])
            nc.sync.dma_start(xt[0][:, 1:XG, :], xp[:, 1:XG, :])
            xt_loaded[0] = True
            nc.sync.dma_start(adT, adp[:])
            nc.sync.dma_start(wv0, wvp[:, 0:XG, :])
            nc.sync.dma_start(tri, trip[:])
            nc.sync.dma_start(gdv, gdp[:])
            nc.sync.dma_start(ident, idp[:])
            nc.sync.dma_start(csd, csp[:])
            csA = csd[:, 0, :]
            csB = csd[:, 1, :]

            ones_col = tri[:, 127:128]   # all-ones [128,1] (MD)
            zb = singles.tile([128, 1], F32)
            nc.vector.memset(zb, 0.0)
            onesf = singles.tile([1, 128], F32)
            nc.vector.memset(onesf, 1.0)

            kT = [persist.tile([128, SA], MD, tag=f"kT{j}", name=f"kT{j}")
                  for j in range(KVL)]
            vv = persist.tile([128, KC + 1, KVL * HD], MD, tag="vv")
            qT = [persist.tile([128, S], MD, tag=f"qT{h}", name=f"qT{h}")
                  for h in range(HL)]
            oTt = [persist.tile([128, 4, S], MD, tag=f"oT{i}", name=f"oT{i}")
                   for i in range((HL + 3) // 4)]

            def oT(h):
                return oTt[h // 4][:, h % 4, :]

            # wo prefetch tiles for the first two n-blocks of stage C
            NB = D // 512
            wopre = [persist.tile([128, HL, 512], MD, tag=f"wopre{i}",
                                  name=f"wopre{i}") for i in range(2)]
            # head-0 Q weight prefetch (DMA'd late in stage A so stage B
            # starts without a weight stall)
            wq0pre = [persist.tile([128, 8, HD], MD, tag=f"wq0pre{i}",
                                   name=f"wq0pre{i}") for i in range(2)]

            if not causal:
                mtt = persist.tile([128, KC, S], F32, tag="mt")
                nc.sync.dma_start(mtt[:, 0:KC // 2, :], mtp[:, 0:KC // 2, :])
                nc.sync.dma_start(mtt[:, KC // 2:KC, :], mtp[:, KC // 2:KC, :])

                def mtsl(kc):
                    return mtt[:, kc, :]

            with tc.tile_pool(name="rpool", bufs=2) as rpool, \
                 tc.tile_pool(name="cpool", bufs=1) as cpool:

                def rope_copy(ps_h):
                    rc = rpool.tile([128, QB], F32, tag="rc", bufs=4)
                    nc.vector.tensor_copy(rc, ps_h)  # frees the psum fast;
                    # DVE, so the Act exp queue stays short
                    return rc

                def rope_rest(rc, dst, hh):
                    # rc rows 0:64 = x0 (even pair elems), 64:128 = x1.
                    # dst[0:64] = x0*cos - x1*sin ; dst[64:128] = x0*sin + x1*cos
                    sl = slice(hh * QB, (hh + 1) * QB)
                    rs = rpool.tile([128, QB], F32, tag="rs", bufs=2)
                    nc.sync.dma_start(rs[0:64, :], rc[64:128, :])
                    nc.sync.dma_start(rs[64:128, :], rc[0:64, :])
                    tm1 = rpool.tile([128, QB], F32, tag="tm1", bufs=1)
                    tm2 = rpool.tile([128, QB], F32, tag="tm2", bufs=1)
                    nc.vector.tensor_mul(tm1, rc, csA[:, sl])
                    nc.vector.tensor_mul(tm2, rs, csB[:, sl])
                    nc.vector.tensor_sub(dst[0:64, sl], tm1[0:64, :], tm2[0:64, :])
                    nc.vector.tensor_add(dst[64:128, sl], tm2[64:128, :],
                                         tm1[64:128, :])

                def emit_rope(ps_h, dst, hh):
                    rope_rest(rope_copy(ps_h), dst, hh)

                # ============ stage A: V + K projections ==================
                with tc.tile_pool(name="wpoolA", bufs=3) as wpoolA, \
                     tc.tile_pool(name="psA", bufs=1, space="PSUM") as psA:
                    vacc = cpool.tile([128, KC, KVL * HD], F32)
                    krc = [[None] * NQH for _ in range(KVL)]
                    psk = [[psA.tile([128, QB], F32, tag=f"psk{j}_{hh}",
                                     name=f"psk{j}_{hh}")
                            for hh in range(NQH)] for j in range(KVL)]
                    pav = psA.tile([A, KVL * HD], F32, tag="pav")
                    pakk = psA.tile([128, KVL * A], F32, tag="pakk")
                    NVB = KO // XG   # V/K stream in x-group-sized blocks

                    def emit_vblock(g):
                        if g == 0:
                            wt = wv0
                        else:
                            wt = wpoolA.tile([128, XG, KVL * HD], MD, tag="wv")
                            nc.sync.dma_start(wt, wvp[:, g * XG:(g + 1) * XG, :])
                        xload(g)
                        for t in range(KC):
                            psv = psA.tile([128, KVL * HD], F32, tag="vproj",
                                           bufs=2)
                            for ci in range(XG):
                                nc.tensor.matmul(
                                    psv[:, :],
                                    xsl(g * XG + ci)[:, t * 128:(t + 1) * 128],
                                    wt[:, ci, :],
                                    start=(ci == 0), stop=(ci == XG - 1))
                            if g == 0 and NVB > 1:
                                nc.scalar.copy(vacc[:, t, :], psv[:, :])
                            elif g < NVB - 1:
                                nc.vector.tensor_add(vacc[:, t, :],
                                                     vacc[:, t, :], psv[:, :])
                            elif NVB > 1:
                                nc.vector.tensor_add(vv[:, t, :],
                                                     vacc[:, t, :], psv[:, :])
                            else:
                                nc.scalar.copy(vv[:, t, :], psv[:, :])
                        for ci in range(XG):
                            c = g * XG + ci
                            nc.tensor.matmul(pav[:, :], adT[:, c, :],
                                             wt[:, ci, :],
                                             start=(c == 0), stop=(c == KO - 1))
                        if g == NVB - 1:
                            nc.scalar.copy(vv[0:A, KC, :], pav[:, :])

                    def emit_kblock(j, g):
                        if g == 0:
                            wt = wk0[j]
                        else:
                            wt = wpoolA.tile([128, XG, HD], MD, tag="wk")
                            nc.sync.dma_start(wt,
                                              wkp[j, :, g * XG:(g + 1) * XG, :])
                        for ci in range(XG):
                            c = g * XG + ci
                            for hh in range(NQH):
                                sl = slice(hh * QB, (hh + 1) * QB)
                                nc.tensor.matmul(
                                    psk[j][hh][:, :], wt[:, ci, :],
                                    xsl(c)[:, sl],
                                    start=(c == 0), stop=(c == KO - 1))
                        for ci in range(XG):
                            c = g * XG + ci
                            # psum start=True poisons the whole 2KB zero
                            # region (bank row): only the first matmul into
                            # the pakk bank may set it; later first-touches
                            # overwrite via the pending-zero bytes
                            nc.tensor.matmul(
                                pakk[:, j * A:(j + 1) * A], wt[:, ci, :],
                                adT[:, c, :],
                                start=(j == 0 and c == 0),
                                stop=(j == KVL - 1 and c == KO - 1),
                                skip_group_check=True)
                        if g == NX - 1:
                            # copy psum out now (frees psk for stage B);
                            # the rope tails are emitted after the psA pool
                            # closes so its exit barrier doesn't chain
                            # stage B behind the whole rope DVE/DMA chain
                            for hh in range(NQH):
                                krc[j][hh] = rope_copy(psk[j][hh])
                            nc.scalar.copy(kT[j][:, S:SA],
                                           pakk[:, j * A:(j + 1) * A])

                    for g in range(NX):
                        if g == 0:
                            # first group: K is paced chunk-by-chunk by the
                            # x DMA (V needs all 4 chunks at once)
                            xload(g)
                            for j in range(KVL):
                                emit_kblock(j, g)
                            emit_vblock(g)
                        else:
                            # V first everywhere else; in the last group its
                            # DVE add-drain then overlaps the K blocks (the
                            # psA pool close waits on all of it)
                            emit_vblock(g)
                            for j in range(KVL):
                                emit_kblock(j, g)
                        if g == NX - 3:
                            for i in range(2):
                                nc.sync.dma_start(wq0pre[i], wqp[0, i])

                # K rope tails (outside psA so its exit barrier is cheap)
                for j in range(KVL):
                    for hh in range(NQH):
                        rope_rest(krc[j][hh], kT[j], hh)

                # ============ stage B: Q projections + attention ==========
                spool_cm = tc.tile_pool(name="spool", bufs=3)
                spool = spool_cm.__enter__()
                wpoolB_cm = tc.tile_pool(name="wpoolB", bufs=3)
                wpoolB = wpoolB_cm.__enter__()
                ps2_cm = tc.tile_pool(name="ps2", bufs=1, space="PSUM")
                ps2 = ps2_cm.__enter__()
                pscur = [ps2]   # attention psum pool (swapped for stage C)
                if True:
                    def emit_qblock(h, b, psq, cis=range(8), hhs=None):
                        if h == 0 and b < 2:
                            wt = wq0pre[b]     # prefetched in stage A
                        elif psq[2] is not None:
                            wt = psq[2]
                        else:
                            wt = wpoolB.tile([128, 8, HD], MD, tag="wq")
                            nc.sync.dma_start(wt, wqp[h, b])
                        psq[2] = wt if cis[-1] != 7 else None
                        for ci in cis:
                            c = b * 8 + ci
                            st, sp = (c == 0), (c == KO - 1)
                            for hh in (range(NQH) if hhs is None else hhs):
                                sl = slice(hh * QB, (hh + 1) * QB)
                                nc.tensor.matmul(
                                    psq[hh][:, :], wt[:, ci, :], xsl(c)[:, sl],
                                    start=st, stop=sp)

                    def attn_open_a(h, qh):
                        """Allocate psum, emit chunk 0 + the adapter scores.
                        The adapter exp queues right behind chunk 0's; the
                        pa-dependent matmuls wait until attn_open_b (the
                        caller interleaves a qblock in between)."""
                        ps = pscur[0]
                        j = h // nrep
                        qs, qe = qh * QB, (qh + 1) * QB
                        if causal:
                            kcs = [kc for kc in range(KC) if kc * 128 < qe]
                        else:
                            kcs = list(range(KC))
                        ot = ps.tile([128, QB], F32, tag="ot", bufs=1)
                        oa = ps.tile([128, QB], F32, tag="oa", bufs=1)
                        # dn ([:, 0:8]) and the transposed reciprocals
                        # rT ([0:8, 8:136]) share one psum bank; the
                        # transpose runs only after the recip has read dn
                        dnrt = ps.tile([128, 2 * NSUB + 128], F32, tag="dn",
                                       bufs=1)
                        st = {"h": h, "qh": qh, "j": j, "qs": qs, "qe": qe,
                              "kcs": kcs, "ot": ot, "oa": oa, "dnrt": dnrt,
                              "ki": 0, "ps": ps}
                        attn_chunk(st)
                        sca = ps.tile([128, QB], F32, tag="scp", bufs=3)
                        nc.tensor.matmul(sca[0:A, :], kT[j][:, S:SA],
                                         qT[h][:, qs:qe], start=True, stop=True)
                        pa = spool.tile([128, QB], MD, tag="pt", bufs=6)
                        nc.scalar.activation(pa[0:A, :], sca[0:A, :], Exp,
                                             bias=zb[0:A, :])
                        st["pa"] = pa
                        return st

                    def attn_open_b(st):
                        """Adapter PV + denominators (pa exp has landed)."""
                        h, j, oa, pa = st["h"], st["j"], st["oa"], st["pa"]
                        nc.tensor.matmul(oa[:, :],
                                         vv[0:A, KC, j * HD:(j + 1) * HD],
                                         pa[0:A, :], start=True, stop=True)
                        dn = st["dnrt"][:, 0:2 * NSUB]
                        # start=False: these land while the bank is pending-
                        # zero from chunk 0's opener, so first touch
                        # overwrites (zero-region semantics)
                        for s in range(NSUB):
                            nc.tensor.matmul(
                                dn[:, NSUB + s:NSUB + s + 1],
                                pa[0:A, s * 128:(s + 1) * 128],
                                gdv[:, h:h + 1], start=False, stop=False,
                                skip_group_check=True)
                        oasb = spool.tile([128, QB], F32, tag="oasb", bufs=2)
                        nc.vector.tensor_copy(oasb, oa)
                        st["oasb"] = oasb

                    def attn_open(h, qh):
                        st = attn_open_a(h, qh)
                        attn_open_b(st)
                        return st

                    def attn_chunk(st, n=1):
                        """Emit the next n score/exp/PV/denominator chunks.
                        After the last chunk, spill ot to SBUF so the psum
                        bank frees without waiting for the norm chain."""
                        h, qh, j = st["h"], st["qh"], st["j"]
                        qs, qe, kcs = st["qs"], st["qe"], st["kcs"]
                        ot, dnrt, ps = st["ot"], st["dnrt"], st["ps"]
                        dn = dnrt[:, 0:2 * NSUB]
                        for _ in range(n):
                            ki = st["ki"]
                            kc = kcs[ki]
                            st["ki"] = ki + 1
                            q0 = max(qs, kc * 128) if causal else qs
                            N = qe - q0
                            scp = ps.tile([128, QB], F32, tag="scp", bufs=3)
                            nc.tensor.matmul(
                                scp[:, 0:N],
                                kT[j][:, kc * 128:(kc + 1) * 128],
                                qT[h][:, q0:qe], start=True, stop=True)
                            pt = spool.tile([128, QB], MD, tag="pt", bufs=6)
                            if causal:
                                nc.scalar.activation(pt[:, 0:N], scp[:, 0:N],
                                                     Exp, bias=zb)
                                if kc * 128 >= qs:  # diagonal chunk
                                    nc.vector.tensor_mul(
                                        pt[:, 0:128], pt[:, 0:128], tri)
                            else:
                                sadd = spool.tile([128, QB], F32, tag="sadd",
                                                  bufs=2)
                                nc.vector.tensor_add(
                                    sadd[:, 0:N], scp[:, 0:N],
                                    mtsl(kc)[:, q0:qe])
                                nc.scalar.activation(pt[:, 0:N], sadd[:, 0:N],
                                                     Exp, bias=zb)
                            nc.tensor.matmul(
                                ot[:, q0 - qs:QB],
                                vv[:, kc, j * HD:(j + 1) * HD],
                                pt[:, 0:N], start=(ki == 0),
                                stop=(ki == len(kcs) - 1))
                            # denominators: probs^T @ ones -> [q,1] columns.
                            # Only the very first matmul into the dn bank
                            # sets start (zero-region semantics); later
                            # columns' first touches overwrite via the
                            # pending-zero bytes.
                            for s in range(NSUB):
                                qa = qs + s * 128
                                if qa < q0:
                                    continue
                                off = qa - q0
                                nc.tensor.matmul(
                                    dn[:, s:s + 1], pt[:, off:off + 128],
                                    ones_col,
                                    start=(ki == 0 and s == 0),
                                    stop=(ki == len(kcs) - 1
                                          and s == NSUB - 1),
                                    skip_group_check=True)
                        if st["ki"] == len(kcs):
                            otsb = spool.tile([128, QB], F32, tag="otsb",
                                              bufs=2)
                            nc.scalar.copy(otsb, ot)
                            st["otsb"] = otsb

                    def attn_norm(st, tail=False):
                        """reciprocal -> per-column transposes -> broadcast
                        -> scale+sum.  Each [128,1] reciprocal column is
                        PE-transposed to a [1,128] row at partition 0 (Pool
                        partition_broadcast requires partition-0 sources),
                        all landing in one [1, 8*128] psum row."""
                        h, qh, dnrt = st["h"], st["qh"], st["dnrt"]
                        qs, qe = st["qs"], st["qe"]
                        ps = st["ps"]
                        rcp = spool.tile([128, 2 * NSUB], F32, tag="rcp",
                                         bufs=2)
                        nc.vector.reciprocal(rcp, dnrt[:, 0:2 * NSUB])
                        # one f32 transpose into the dn bank (runs after the
                        # recip read), then DVE -> SBUF, then a tiny
                        # SBUF->SBUF DMA flattens [8,128] onto partition 0
                        # so the Pool broadcasts have legal sources
                        rT = dnrt[0:2 * NSUB, 2 * NSUB:2 * NSUB + 128]
                        nc.tensor.matmul(rT, rcp, ident, is_transpose=True,
                                         skip_group_check=True)
                        rTs = spool.tile([2 * NSUB, 128], F32, tag="rTs",
                                         bufs=2)
                        nc.vector.tensor_copy(rTs, rT)
                        if tail:
                            # rank-1 PE broadcasts straight from the rTs
                            # rows: no DMA-FIFO wait, no Pool chain; the PE
                            # is otherwise idle at the pipeline tail
                            rp1 = ps.tile([128, QB], F32, tag="scp", bufs=3)
                            rp2 = ps.tile([128, QB], F32, tag="scp", bufs=3)
                            for s in range(NSUB):
                                nc.tensor.matmul(
                                    rp1[:, s * 128:(s + 1) * 128], onesf,
                                    rTs[s:s + 1, :], start=(s == 0),
                                    stop=(s == NSUB - 1),
                                    skip_group_check=True)
                            for s in range(NSUB):
                                nc.tensor.matmul(
                                    rp2[:, s * 128:(s + 1) * 128], onesf,
                                    rTs[NSUB + s:NSUB + s + 1, :],
                                    start=(s == 0), stop=(s == NSUB - 1),
                                    skip_group_check=True)
                            tq1 = spool.tile([128, QB], F32, tag="tq1",
                                             bufs=1)
                            nc.vector.tensor_mul(tq1, st["otsb"], rp1)
                            tq2 = spool.tile([128, QB], F32, tag="tq2",
                                             bufs=1)
                            nc.vector.tensor_mul(tq2, st["oasb"], rp2)
                            nc.gpsimd.tensor_add(oT(h)[:, qs:qe], tq1, tq2)
                            return
                        rfs = spool.tile([1, 2 * NSUB * 128], F32, tag="rfs",
                                         bufs=2)
                        hw = NSUB * 128
                        nc.sync.dma_start(rfs[0:1, 0:hw], rTs[0:NSUB, :])
                        nc.sync.dma_start(rfs[0:1, hw:2 * hw], rTs[NSUB:, :])
                        rtb = spool.tile([128, QB], F32, tag="rtb", bufs=2)
                        rab = spool.tile([128, QB], F32, tag="rab", bufs=2)
                        for s in range(NSUB):
                            nc.gpsimd.partition_broadcast(
                                rtb[:, s * 128:(s + 1) * 128],
                                rfs[0:1, s * 128:(s + 1) * 128])
                            nc.gpsimd.partition_broadcast(
                                rab[:, s * 128:(s + 1) * 128],
                                rfs[0:1, (NSUB + s) * 128:(NSUB + s + 1) * 128])
                        # tq2 + the final add run on Pool (idle) so the DVE
                        # queue (rope + spills) never gates the oT writes
                        tq1 = spool.tile([128, QB], F32, tag="tq1", bufs=1)
                        nc.vector.tensor_mul(tq1, st["otsb"], rtb)
                        tq2 = spool.tile([128, QB], F32, tag="tq2", bufs=1)
                        nc.gpsimd.tensor_mul(tq2, st["oasb"], rab)
                        nc.gpsimd.tensor_add(oT(h)[:, qs:qe], tq1, tq2)
                        if KDBG and h == 0 and qh == 0:
                            dcp = spool.tile([128, 8], F32, tag="dcp", bufs=1)
                            nc.vector.tensor_copy(dcp, dnrt)
                            nc.sync.dma_start(dbgp[:, 0:8], dcp)
                            rfc = spool.tile([1, 1024], F32, tag="rfc", bufs=1)
                            nc.vector.tensor_copy(rfc, rfs)
                            nc.sync.dma_start(dbgp[0:1, 1024:2048], rfc)
                            rtc = spool.tile([128, QB], F32, tag="rtc", bufs=1)
                            nc.vector.tensor_copy(rtc, rtb)
                            nc.sync.dma_start(dbgp[:, 2048:2560], rtc)
                            nc.sync.dma_start(dbgp[:, 2560:3072], st["otsb"])
                            nc.sync.dma_start(dbgp[:, 3072:3584], tq1)

                    # last-head weight tiles, prefetched a head early
                    wtN = [wpoolB.tile([128, 8, HD], MD, tag=f"wqN{b}",
                                       bufs=1, name=f"wqN{b}")
                           for b in range(4)]

                    # per-head software pipeline: project h, attend h-1
                    for h in range(HL - 1):
                        if h == HL - 2:
                            for b in range(4):
                                nc.sync.dma_start(wtN[b], wqp[HL - 1, b])
                        if h == 2:
                            # prefetch wo for stage C's first two n-blocks
                            # (DMA is idle here; stage C is far away)
                            for i in range(2):
                                for wpi in range(0, HL, 4):
                                    nc.sync.dma_start(
                                        wopre[i][:, wpi:wpi + 4, :],
                                        wop[:, wpi:wpi + 4,
                                            i * 512:(i + 1) * 512])
                        psq = [ps2.tile([128, QB], F32, tag=f"psq{hh}",
                                        bufs=1, name=f"psq{h}_{hh}")
                               for hh in range(NQH)] + [None]
                        emit_qblock(h, 0, psq)
                        emit_qblock(h, 1, psq)
                        if h > 0:
                            st0 = attn_open_a(h - 1, 0)
                        emit_qblock(h, 2, psq, range(4))
                        if h > 0:
                            attn_open_b(st0)
                            attn_chunk(st0, len(st0["kcs"]) - 1)
                        emit_qblock(h, 2, psq, range(4, 8))
                        if h > 0:
                            attn_norm(st0)
                            st1 = attn_open_a(h - 1, 1)
                        emit_qblock(h, 3, psq, range(4))
                        if h > 0:
                            attn_open_b(st1)
                            attn_chunk(st1, len(st1["kcs"]) - 1)
                        emit_qblock(h, 3, psq, range(4, 8))
                        if h > 0:
                            attn_norm(st1)
                        for hh in range(NQH):
                            emit_rope(psq[hh], qT[h], hh)

                    # last head hN: project hh-halves separately so its rope
                    # lands early; its qh=0 attention runs here in stage B,
                    # hidden under the hh=1 projection pass
                    hN = HL - 1
                    psq = [ps2.tile([128, QB], F32, tag=f"psq{hh}",
                                    bufs=1, name=f"psqN_{hh}")
                           for hh in range(NQH)] + [None]

                    def qblockN(b, hh):
                        for ci in range(8):
                            c = b * 8 + ci
                            nc.tensor.matmul(
                                psq[hh][:, :], wtN[b][:, ci, :],
                                xsl(c)[:, hh * QB:(hh + 1) * QB],
                                start=(c == 0), stop=(c == KO - 1))

                    qblockN(0, 0)
                    st0 = attn_open_a(hN - 1, 0)
                    qblockN(1, 0)
                    attn_open_b(st0)
                    attn_chunk(st0, len(st0["kcs"]) - 1)
                    qblockN(2, 0)
                    attn_norm(st0, tail=True)
                    st1 = attn_open_a(hN - 1, 1)
                    qblockN(3, 0)
                    attn_open_b(st1)
                    attn_chunk(st1, 3)
                    attn_chunk(st1, len(st1["kcs"]) - 4)
                    qblockN(0, 1)
                    attn_norm(st1, tail=True)
                    emit_rope(psq[0], qT[hN], 0)
                    qblockN(1, 1)
                    stN0 = attn_open_a(hN, 0)
                    qblockN(2, 1)
                    attn_open_b(stN0)
                    attn_chunk(stN0, len(stN0["kcs"]) - 1)
                    qblockN(3, 1)
                    attn_norm(stN0, tail=True)
                    emit_rope(psq[1], qT[hN], 1)

                # close stage-B psum pool so stage C can reuse its banks
                ps2_cm.__exit__(None, None, None)
                wpoolB_cm.__exit__(None, None, None)

                # ===== stage C: last head's attention + output projection ==
                with tc.tile_pool(name="wopool", bufs=2) as wopool, \
                     tc.tile_pool(name="obpool", bufs=2) as obpool, \
                     tc.tile_pool(name="ps3", bufs=1, space="PSUM") as ps3:
                    pscur[0] = ps3
                    MB = S // 128

                    ob_cur = [None]

                    def emit_wo(n, won, ms, single_dma=False):
                        for m in ms:
                            pso = ps3.tile([128, 512], F32, tag="wo", bufs=2)
                            for hh2 in range(HL):
                                nc.tensor.matmul(
                                    pso,
                                    oT(hh2)[:, m * 128:(m + 1) * 128],
                                    won[:, hh2, :],
                                    start=(hh2 == 0), stop=(hh2 == HL - 1))
                            if single_dma:
                                ob = obpool.tile([128, 2, 512], MD,
                                                 tag="ob", name=f"ob{n}_{m}")
                                nc.scalar.copy(ob[:, 0, :], pso)
                                nc.sync.dma_start(
                                    outp[:, m:m + 1,
                                         n * 512:(n + 1) * 512],
                                    ob[:, 0:1, :])
                                continue
                            if m % 2 == 0:
                                ob_cur[0] = obpool.tile([128, 2, 512], MD,
                                                        tag="ob",
                                                        name=f"ob{n}_{m}")
                            ob = ob_cur[0]
                            nc.scalar.copy(ob[:, m % 2, :], pso)
                            if m % 2 == 1:
                                nc.sync.dma_start(
                                    outp[:, m - 1:m + 1,
                                         n * 512:(n + 1) * 512], ob)

                    # interleave the last attention block (hN, qh=1) with
                    # the first WO n-block: m0-3 only need oT(hN) qh=0,
                    # which stage B already produced
                    emit_wo(0, wopre[0], range(0, 1))
                    stc1 = attn_open_a(hN, 1)
                    emit_wo(0, wopre[0], range(1, 2))
                    attn_open_b(stc1)
                    attn_chunk(stc1, 3)
                    emit_wo(0, wopre[0], range(2, 3))
                    attn_chunk(stc1, len(stc1["kcs"]) - 4)
                    emit_wo(0, wopre[0], range(3, 4))
                    attn_norm(stc1, tail=True)
                    # n=1's m0-3 only need the qh=0 norms: they run while
                    # the last norm chain (Pool/DVE) finishes
                    emit_wo(1, wopre[1], range(0, 4))
                    emit_wo(0, wopre[0], range(4, MB))
                    emit_wo(1, wopre[1], range(4, MB))
                    for n in range(2, NB):
                        won = wopool.tile([128, HL, 512], MD, tag="won")
                        for wpi in range(0, HL, 4):
                            nc.sync.dma_start(
                                won[:, wpi:wpi + 4, :],
                                wop[:, wpi:wpi + 4,
                                    n * 512:(n + 1) * 512])
                        emit_wo(n, won, range(MB))

                spool_cm.__exit__(None, None, None)

    nc.compile()
    nc.finalize()
    return nc


def get_program(KO, S, HL, KVL, causal, mm):
    key = (KO, S, HL, KVL, causal, mm)
    if key not in _PROG_CACHE:
        _PROG_CACHE[key] = build_program(KO, S, HL, KVL, causal, mm)
    return _PROG_CACHE[key]


# --------------------------------------------------------------------------
# host-side sharding / layout prep
# --------------------------------------------------------------------------

_EVEN_FIRST = np.concatenate([np.arange(0, HD, 2), np.arange(1, HD, 2)])


def is_causal_mask(mask):
    S = mask.shape[-1]
    m = np.asarray(mask).reshape(S, S)
    iu = np.triu_indices(S, 1)
    il = np.tril_indices(S)
    return bool(np.all(m[il] == 0.0) and np.all(m[iu] <= -1e8))


def _np_md(mm):
    if mm == "bf16":
        import ml_dtypes
        return ml_dtypes.bfloat16
    return np.float32


def prep_core_inputs(core, G, x, wq, wk, wv, wo, adapter, gate,
                     freqs_cos, freqs_sin, mask, causal, mm=None):
    """Build the input dict for one core = (batch b, head-group g)."""
    mm = MM_MODE if mm is None else mm
    B, S, D = x.shape
    H = gate.shape[1]
    hd = wq.shape[1] // H
    KV = wk.shape[1] // hd
    KO = D // 128
    KC = S // 128
    HL, KVL = H // G, KV // G
    b, g = core // G, core % G
    hsl = slice(g * HL, (g + 1) * HL)
    ksl = slice(g * KVL, (g + 1) * KVL)
    idx = _EVEN_FIRST
    f32 = np.float32
    md = _np_md(mm)

    def c(a, dt=None):
        return np.ascontiguousarray(a, dtype=dt if dt is not None else md)

    xp = c(x[b].T.reshape(KO, 128, S).transpose(1, 0, 2))
    wq4 = wq.reshape(D, H, hd)[:, hsl][:, :, idx] * np.float32(1.0 / np.sqrt(hd))
    wqp = c(wq4.reshape(KO, 128, HL, hd).transpose(2, 1, 0, 3)
            .reshape(HL, 128, KO // 8, 8 * hd).transpose(0, 2, 1, 3))
    wk4 = wk.reshape(D, KV, hd)[:, ksl][:, :, idx]
    wkp = c(wk4.reshape(KO, 128, KVL, hd).transpose(2, 1, 0, 3))
    wv4 = wv.reshape(D, KV, hd)[:, ksl]
    wvp = c(wv4.reshape(KO, 128, KVL * hd).transpose(1, 0, 2))
    wos = wo[g * HL * hd:(g + 1) * HL * hd]
    wop = c(wos.reshape(HL, hd, D).transpose(1, 0, 2))
    adp = c(adapter[0].T.reshape(KO, 128, A).transpose(1, 0, 2))
    # cos^T / sin^T, each duplicated across both partition halves
    ct = np.asarray(freqs_cos, dtype=f32).T      # [64, S]
    st = np.asarray(freqs_sin, dtype=f32).T
    csp = np.empty((128, 2, S), f32)
    csp[0:64, 0] = ct
    csp[64:128, 0] = ct
    csp[0:64, 1] = st
    csp[64:128, 1] = st
    tri = c(np.triu(np.ones((128, 128), dtype=f32)))
    idp = np.eye(128, dtype=f32)
    gth = np.tanh(np.asarray(gate[0, hsl, 0, 0], dtype=np.float64))
    gdp = c(np.broadcast_to((1.0 / gth).astype(f32), (A, HL)))
    inp = {"xp": xp, "wqp": wqp, "wkp": wkp, "wvp": wvp, "wop": wop,
           "adp": adp, "csp": csp, "trip": tri, "idp": idp, "gdp": gdp}
    if not causal:
        mt = np.asarray(mask).reshape(S, S).T  # [keys, q]
        inp["mtp"] = c(mt.reshape(KC, 128, S).transpose(1, 0, 2), f32)
    return inp


# --------------------------------------------------------------------------
# entry point
# --------------------------------------------------------------------------

def kernel(x, wq, wk, wv, wo, adapter, gate, freqs_cos, freqs_sin, mask,
           _trace=False):
    x, wq, wk, wv, wo, adapter, gate, freqs_cos, freqs_sin, mask = (
        np.asarray(a) for a in
        (x, wq, wk, wv, wo, adapter, gate, freqs_cos, freqs_sin, mask))
    B, S, D = x.shape
    H = gate.shape[1]
    hd = wq.shape[1] // H
    KV = wk.shape[1] // hd
    G = 8 // B                      # head groups per batch over 8 cores
    HL, KVL = H // G, KV // G
    KO = D // 128

    causal = is_causal_mask(mask)
    nc = get_program(KO, S, HL, KVL, causal, MM_MODE)

    in_maps = [prep_core_inputs(core, G, x, wq, wk, wv, wo, adapter, gate,
                                freqs_cos, freqs_sin, mask, causal)
               for core in range(8)]
    res = run_bass_kernel_spmd(nc, in_maps, core_ids=list(range(8)),
                               trace=_trace)
    out = np.zeros((B, S, D), np.float32)
    for core in range(8):
        b = core // G
        r = res.results[core]
        # device layout: [128, S//128, D] partition-major
        out[b] += np.asarray(r["out"], dtype=np.float32)\
            .transpose(1, 0, 2).reshape(S, D)
    if _trace:
        kernel._last_result = res
    return out
